# revision 17
# baseline (speedup 1.0000x reference)
"""Trainium2 Bass kernel for a 2-layer GATv2 encoder (nn_CG_GNN_Encoder).

kernel(**inputs) takes full inputs (x [20000,512] f32, edge_index [2,320000]
int64, weights) and returns the full [20000, 512] f32 output, across 8 cores.

v3 design (per core, dst-node sharded):
  - Host: balance dst nodes into 8 cores x 20 blocks x 125 nodes; per-block
    edge lists padded to e_blk; one-hot scatter matrices in BOTH orientations
    (edge-major `oh` for value aggregation, dst-major `ohT` for broadcasting
    dst features to edges); |att| magnitudes folded into Wl/Wr columns with
    pos-att columns ordered before neg-att per head.
  - Phase A per layer: x chunks DMA-transposed, 8 matmuls per 125-node tile
    -> xl/xr [., 512], bias added during PSUM evacuation; xl stored to DRAM
    and AllGathered in 4 chunks (overlapping phase A); xr stays local.
  - Edge phase per block: ONE batched indirect gather (xl[src]) split across
    2 SWDGE queues; xr[dst] broadcast on the tensor engine (ohT matmul) with
    xl accumulated via identity matmul; LeakyReLU applied by the scalar
    engine during PSUM->SBUF evacuation.  Per-head logits = pos-column sum
    minus neg-column sum (DVE reduces), p = exp(logit); p duplicated into
    adjacent column pairs so the value multiply runs in the DVE packed 2x
    mode; one-hot matmuls accumulate values + denominators in PSUM;
    normalize, ELU between layers.  Layer-1 output unscale/bias on host.
"""

import numpy as np
from ml_dtypes import bfloat16

import concourse.bacc as bacc
import concourse.bass as bass
import concourse.mybir as mybir
import concourse.tile as tile
from concourse.bass_utils import run_bass_kernel_spmd

F32 = mybir.dt.float32
BF16 = mybir.dt.bfloat16
I16 = mybir.dt.int16
AX = mybir.AxisListType
OP = mybir.AluOpType
ACT = mybir.ActivationFunctionType

N = 20000
H = 4
C = 128
IN = 512
HC = H * C            # 512
NEG = 0.2
NCORES = 8
NSH = N // NCORES     # 2500
DBLK = 125
NBLK = NSH // DBLK    # 20
NPAD = NBLK * 128     # 2560 padded rows (tile t at rows 128t..128t+124)
AGCH = 250            # AllGather chunk rows per core (10 chunks per layer)
ATT_EPS = 1e-10


# ----------------------------------------------------------------------------
# Host-side preprocessing
# ----------------------------------------------------------------------------

def _preprocess_graph(edge_index):
    src = np.concatenate([edge_index[0], np.arange(N, dtype=np.int64)])
    dst = np.concatenate([edge_index[1], np.arange(N, dtype=np.int64)])
    deg = np.bincount(dst, minlength=N)

    nbins = NCORES * NBLK
    order = np.argsort(-deg, kind="stable")
    import heapq
    bin_load = np.zeros(nbins, np.int64)
    bin_fill = np.zeros(nbins, np.int64)
    assign = np.zeros(N, np.int64)
    heap = [(0, b) for b in range(nbins)]
    heapq.heapify(heap)
    for nid in order:
        while True:
            load, b = heapq.heappop(heap)
            if bin_fill[b] < DBLK:
                break
        assign[nid] = b
        bin_fill[b] += 1
        bin_load[b] = load + deg[nid]
        if bin_fill[b] < DBLK:
            heapq.heappush(heap, (bin_load[b], b))

    perm = np.argsort(assign * N + np.arange(N), kind="stable")
    inv_perm = np.empty(N, np.int64)
    inv_perm[perm] = np.arange(N)

    e_bin = assign[dst]
    e_dst_pos = inv_perm[dst]
    e_src_pos = inv_perm[src]
    max_per_bin = int(np.bincount(e_bin, minlength=nbins).max())
    e_blk = -(-max_per_bin // 128) * 128
    S = e_blk // 128

    order_e = np.argsort(e_bin, kind="stable")
    eb = e_bin[order_e]
    starts = np.searchsorted(eb, np.arange(nbins))
    ends = np.searchsorted(eb, np.arange(nbins), side="right")

    E16 = e_blk // 16
    src16 = np.zeros((NCORES, NBLK, 16, E16), np.int16)
    onehot = np.zeros((NCORES, 128, NBLK, S, DBLK), bfloat16)
    onehotT = np.zeros((NCORES, DBLK, NBLK, S, 128), bfloat16)

    for b in range(nbins):
        core, blk = divmod(b, NBLK)
        sel = order_e[starts[b]:ends[b]]
        n = len(sel)
        pos = np.arange(n)
        d_loc = e_dst_pos[sel] % DBLK
        # xl_full is chunk-major: AllGather chunk c (625 rows per core) is
        # contiguous as [8 cores, 625].  Map src position -> xl_full row.
        sp_ = e_src_pos[sel]
        s_core, s_r = sp_ // NSH, sp_ % NSH
        src_row = (s_r // AGCH) * (NCORES * AGCH) + s_core * AGCH + (s_r % AGCH)
        src16[core, blk, pos % 16, pos // 16] = src_row
        onehot[core, pos % 128, blk, pos // 128, d_loc] = 1.0
        onehotT[core, d_loc, blk, pos // 128, pos % 128] = 1.0

    per_core = []
    for core in range(NCORES):
        s16 = src16[core].transpose(1, 0, 2).reshape(16, -1)
        per_core.append(dict(
            src_idx=np.tile(s16, (8, 1)).copy(),
            onehot=onehot[core].reshape(128, -1).copy(),
            onehotT=onehotT[core].reshape(DBLK, -1).copy(),
        ))
    return per_core, dict(e_blk=e_blk, perm=perm)


def _prep_weights(inputs):
    """Per layer: permute columns pos-att-first per head, scale columns by
    max(|att|, eps).  logit = sum_pos lrelu(col) - sum_neg lrelu(col)."""
    out = {}
    npos = []
    col_perms = []
    invs = []
    for l in range(2):
        att = np.asarray(inputs[f"att{l}"], np.float32)
        cols = []
        np_l = []
        for h in range(H):
            pos = np.where(att[h] >= 0)[0]
            neg = np.where(att[h] < 0)[0]
            cols.append(h * C + np.concatenate([pos, neg]))
            np_l.append(len(pos))
        cols = np.concatenate(cols)
        absa = np.maximum(np.abs(att.reshape(HC)[cols]), ATT_EPS)
        col_perms.append(cols)
        npos.append(np_l)
        invs.append((1.0 / absa).astype(np.float32))

        Wl = np.asarray(inputs[f"Wl{l}"], np.float32)
        Wr = np.asarray(inputs[f"Wr{l}"], np.float32)
        bl = np.asarray(inputs[f"bl{l}"], np.float32)
        br = np.asarray(inputs[f"br{l}"], np.float32)
        if l == 1:
            Wl = Wl[col_perms[0], :]
            Wr = Wr[col_perms[0], :]
        out[f"wl{l}"] = (Wl[:, cols] * absa[None, :]).astype(bfloat16)
        out[f"wr{l}"] = (Wr[:, cols] * absa[None, :]).astype(bfloat16)
        aux = np.zeros((4, HC), np.float32)
        aux[0] = bl[cols] * absa
        aux[1] = br[cols] * absa
        aux[2] = invs[l]
        aux[3] = np.asarray(inputs[f"bias{l}"], np.float32)[cols]
        out[f"aux{l}"] = aux.astype(bfloat16)
    return out, npos, col_perms, invs


# ----------------------------------------------------------------------------
# Device kernel
# ----------------------------------------------------------------------------

def _build(e_blk, npos):
    S = e_blk // 128
    E16 = e_blk // 16
    S0 = (S + 1) // 2          # chunks handled by queue 0
    n0 = S0 * 128
    n1 = e_blk - n0
    nc = bacc.Bacc("TRN2", target_bir_lowering=False, debug=False,
                   num_devices=NCORES, num_swdge_queues=2)

    x_in = nc.dram_tensor("x_pad", [NPAD, IN], BF16, kind="ExternalInput")
    wl_d = [nc.dram_tensor(f"wl{l}", [IN, HC], BF16, kind="ExternalInput")
            for l in range(2)]
    wr_d = [nc.dram_tensor(f"wr{l}", [IN, HC], BF16, kind="ExternalInput")
            for l in range(2)]
    aux_d = [nc.dram_tensor(f"aux{l}", [4, HC], BF16, kind="ExternalInput")
             for l in range(2)]
    srcidx_d = nc.dram_tensor("src_idx", [128, NBLK * E16], I16,
                              kind="ExternalInput")
    oh_d = nc.dram_tensor("onehot", [128, NBLK * S * DBLK], BF16,
                          kind="ExternalInput")
    ohT_d = nc.dram_tensor("onehotT", [DBLK, NBLK * S * 128], BF16,
                           kind="ExternalInput")
    out_d = nc.dram_tensor("out", [NSH, HC], F32, kind="ExternalOutput")

    from concourse.masks import make_identity

    with tile.TileContext(nc) as tc:
        with tc.tile_pool(name="dram", bufs=1, space="DRAM") as dram, \
             tc.tile_pool(name="const", bufs=1) as cp, \
             tc.tile_pool(name="pha", bufs=3) as wp, \
             tc.tile_pool(name="gath", bufs=2) as gp, \
             tc.tile_pool(name="sm", bufs=2) as sp, \
             tc.tile_pool(name="psum", bufs=2, space="PSUM") as pp:

            xl_sh = [dram.tile([NSH, HC], BF16, name=f"xl_sh{l}")
                     for l in range(2)]
            xr_dr = [dram.tile([NSH, HC], BF16, name=f"xr_dr{l}")
                     for l in range(2)]
            xl_full = [dram.tile([N, HC], BF16, name=f"xl_full{l}")
                       for l in range(2)]
            h_pad = dram.tile([NPAD, HC], BF16, name="h_pad")

            ident = cp.tile([128, 128], BF16, name="ident")
            make_identity(nc, ident[:])

            si_t = cp.tile([128, NBLK * E16], I16, name="si_t")
            nc.scalar.dma_start(out=si_t[:], in_=srcidx_d[:])

            # weights + aux broadcast tiles (both layers, resident)
            wl_t, wr_t, aux_b = [], [], []
            for l in range(2):
                wlt = cp.tile([128, 4, HC], BF16, name=f"wl_t{l}")
                wrt = cp.tile([128, 4, HC], BF16, name=f"wr_t{l}")
                for k in range(4):
                    nc.scalar.dma_start(out=wlt[:, k, :],
                                        in_=wl_d[l][k * 128:(k + 1) * 128, :])
                    nc.scalar.dma_start(out=wrt[:, k, :],
                                        in_=wr_d[l][k * 128:(k + 1) * 128, :])
                wl_t.append(wlt)
                wr_t.append(wrt)
                rows = []
                for r in range(4):
                    if l == 1 and r >= 2:
                        rows.append(None)
                        continue
                    row = cp.tile([1, HC], BF16, name=f"ar{l}{r}")
                    nc.scalar.dma_start(out=row[:], in_=aux_d[l][r:r + 1, :])
                    bc = cp.tile([128, HC], BF16, name=f"ab{l}{r}")
                    nc.gpsimd.partition_broadcast(bc[:], row[:])
                    rows.append(bc)
                aux_b.append(rows)

            # ---- emit helpers ---------------------------------------------
            def phase_a_group(l, g):
                """Tiles 2g..2g+1 (256 padded rows) of phase A for layer l."""
                src_pad = x_in if l == 0 else h_pad
                xT = wp.tile([128, 4, 256], BF16, name="xT", tag="xT")
                for k in range(4):
                    nc.sync.dma_start_transpose(
                        out=xT[:, k, :],
                        in_=src_pad[g * 256:(g + 1) * 256,
                                    k * 128:(k + 1) * 128])
                for t in range(2 * g, 2 * g + 2):
                    off = (t - 2 * g) * 128
                    ps_xl = pp.tile([DBLK, HC], F32, name="ps_xl",
                                    tag="ps_sh", bufs=2)
                    for k in range(4):
                        nc.tensor.matmul(
                            out=ps_xl[:], lhsT=xT[:, k, off:off + DBLK],
                            rhs=wl_t[l][:, k, :],
                            start=(k == 0), stop=(k == 3))
                    xl_o = wp.tile([DBLK, HC], BF16, name="xl_o", tag="xl_o")
                    nc.vector.tensor_add(out=xl_o[:], in0=ps_xl[:],
                                         in1=aux_b[l][0][:DBLK, :])
                    rows = slice(t * DBLK, (t + 1) * DBLK)
                    nc.sync.dma_start(out=xl_sh[l][rows, :], in_=xl_o[:])
                    ps_xr = pp.tile([DBLK, HC], F32, name="ps_xr",
                                    tag="ps_sh", bufs=2)
                    for k in range(4):
                        nc.tensor.matmul(
                            out=ps_xr[:], lhsT=xT[:, k, off:off + DBLK],
                            rhs=wr_t[l][:, k, :],
                            start=(k == 0), stop=(k == 3))
                    xr_o = wp.tile([DBLK, HC], BF16, name="xr_o", tag="xr_o")
                    nc.vector.tensor_add(out=xr_o[:], in0=ps_xr[:],
                                         in1=aux_b[l][1][:DBLK, :])
                    nc.sync.dma_start(out=xr_dr[l][rows, :], in_=xr_o[:])

            def ag_chunk(l, c):
                nc.gpsimd.collective_compute(
                    "AllGather", OP.bypass,
                    replica_groups=[list(range(NCORES))],
                    ins=[xl_sh[l][c * AGCH:(c + 1) * AGCH, :]],
                    outs=[xl_full[l][c * NCORES * AGCH:
                                     (c + 1) * NCORES * AGCH, :]],
                )

            # halves: half 0 = s in [0, S0), half 1 = s in [S0, S)
            halves = [(0, S0), (S0, S)]

            def edge_load(l, b):
                """Gathers + streams for block b (both halves)."""
                io0 = b * E16
                xl_gh = []
                for hf, (sa, sb_) in enumerate(halves):
                    nh = (sb_ - sa) * 128
                    xg = gp.tile([128, S0, HC], BF16, name=f"xl_g{hf}",
                                 tag=f"xl_g{hf}")
                    nc.gpsimd.dma_gather(
                        out_ap=xg[:, 0:sb_ - sa, :], in_ap=xl_full[l][:],
                        idxs_ap=si_t[:, io0 + sa * 8:io0 + sa * 8 + nh // 16],
                        num_idxs=nh, num_idxs_reg=nh, elem_size=HC,
                        single_packet=False, queue_num=hf)
                    xl_gh.append(xg)

                oh_b = gp.tile([128, S, DBLK], BF16, name="oh_b", tag="oh_b")
                nc.sync.dma_start(
                    out=oh_b[:],
                    in_=oh_d[:, b * S * DBLK:(b + 1) * S * DBLK])
                ohT_b = gp.tile([DBLK, S, 128], BF16, name="ohT_b",
                                tag="ohT_b")
                nc.sync.dma_start(
                    out=ohT_b[:],
                    in_=ohT_d[:, b * S * 128:(b + 1) * S * 128])
                xr_b = gp.tile([DBLK, HC], BF16, name="xr_b", tag="xr_b")
                nc.sync.dma_start(
                    out=xr_b[:], in_=xr_dr[l][b * DBLK:(b + 1) * DBLK, :])
                return xl_gh, oh_b, ohT_b, xr_b

            def edge_mm(l, b, ld):
                """t = xr[dst] + xl[src] matmuls into PSUM."""
                xl_gh, oh_b, ohT_b, xr_b = ld
                ps_lists = [[], []]
                for hf, (sa, sb_) in enumerate(halves):
                    for s2 in range(sa, sb_, 2):
                        w = min(2, sb_ - s2)
                        ps_t = pp.tile([128, 2, HC], F32, name="ps_t",
                                       tag="ps_t", bufs=3)
                        for s in range(s2, s2 + w):
                            nc.tensor.matmul(out=ps_t[:, s - s2, :],
                                             lhsT=ohT_b[:, s, :], rhs=xr_b[:],
                                             start=True, stop=False)
                            nc.tensor.matmul(
                                out=ps_t[:, s - s2, :], lhsT=ident[:],
                                rhs=xl_gh[hf][:, s - sa, :],
                                start=False, stop=True)
                        ps_lists[hf].append((s2, w, ps_t))
                return xl_gh, oh_b, ps_lists

            def edge_evac(l, b, st):
                """ACT LeakyReLU evacuation PSUM -> bf16 SBUF, per half."""
                xl_gh, oh_b, ps_lists = st
                t_lrh = []
                for hf, (sa, sb_) in enumerate(halves):
                    t_lr = gp.tile([128, S0, HC], BF16, name=f"t_lr{hf}",
                                   tag=f"t_lr{hf}")
                    for s2, w, ps_t in ps_lists[hf]:
                        nc.scalar.activation(
                            out=t_lr[:, s2 - sa:s2 - sa + w, :],
                            in_=ps_t[:, 0:w, :], func=ACT.Prelu, alpha=NEG)
                    t_lrh.append(t_lr)
                return t_lrh

            def edge_B1(l, b, st, t_lrh):
                """Per half: logit reduces + exp + value multiply."""
                xl_gh, oh_b, ps_lists = st
                xa = gp.tile([128, S, 2, 258], BF16, name="xa", tag="xa")
                for hf, (sa, sb_) in enumerate(halves):
                    ns = sb_ - sa
                    t_lr = t_lrh[hf]
                    lg_pn = sp.tile([128, 2, S0, H], F32, name=f"lg_pn{hf}",
                                    tag=f"lg_pn{hf}")
                    for h in range(H):
                        np_h = npos[l][h]
                        lo, mid, hi = h * C, h * C + np_h, (h + 1) * C
                        if np_h > 0:
                            nc.vector.tensor_reduce(
                                out=lg_pn[:, 0, 0:ns, h],
                                in_=t_lr[:, 0:ns, lo:mid],
                                axis=AX.X, op=OP.add)
                        else:
                            nc.vector.memset(lg_pn[:, 0, 0:ns, h], 0.0)
                        if np_h < C:
                            nc.vector.tensor_reduce(
                                out=lg_pn[:, 1, 0:ns, h],
                                in_=t_lr[:, 0:ns, mid:hi],
                                axis=AX.X, op=OP.add)
                        else:
                            nc.vector.memset(lg_pn[:, 1, 0:ns, h], 0.0)
                    lg = sp.tile([128, S0, H], F32, name=f"lg{hf}",
                                 tag=f"lg{hf}")
                    nc.vector.tensor_tensor(out=lg[:, 0:ns, :],
                                            in0=lg_pn[:, 0, 0:ns, :],
                                            in1=lg_pn[:, 1, 0:ns, :],
                                            op=OP.subtract)
                    # p = exp(lg): denominator cols + duplicated pairs
                    nc.scalar.activation(
                        out=xa[:, sa:sb_, :, 256:258],
                        in_=lg[:, 0:ns, :].rearrange(
                            "p s (a b) -> p s a b", a=2),
                        func=ACT.Exp)
                    p_dup = sp.tile([128, S0, H, 2], BF16, name=f"p_dup{hf}",
                                    tag=f"p_dup{hf}")
                    nc.scalar.activation(out=p_dup[:, 0:ns, :, 0],
                                         in_=lg[:, 0:ns, :], func=ACT.Exp)
                    nc.scalar.activation(out=p_dup[:, 0:ns, :, 1],
                                         in_=lg[:, 0:ns, :], func=ACT.Exp)
                    # xa = xl * p (packed 2x)
                    for h in range(H):
                        hp, hh = divmod(h, 2)
                        nc.vector.tensor_tensor(
                            out=xa[:, sa:sb_, hp,
                                   hh * 128:(hh + 1) * 128].rearrange(
                                "p s (pr two) -> p s pr two", two=2),
                            in0=xl_gh[hf][:, 0:ns,
                                          h * 128:(h + 1) * 128].rearrange(
                                "p s (pr two) -> p s pr two", two=2),
                            in1=p_dup[:, 0:ns, h, None, :]
                                .to_broadcast([128, ns, 64, 2]),
                            op=OP.mult)
                return xa

            def edge_B2(l, b, st, xa):
                """Value one-hot matmuls + normalize; ELU/store deferred."""
                xl_gh, oh_b, ps_lists = st
                ps_o = [pp.tile([DBLK, HC], F32, name=f"ps_o{hp}",
                                tag="ps_sh", bufs=2) for hp in range(2)]
                for s in range(S):
                    for hp in range(2):
                        nc.tensor.matmul(out=ps_o[hp][:, 0:258],
                                         lhsT=oh_b[:, s, :],
                                         rhs=xa[:, s, hp, :],
                                         start=(s == 0), stop=(s == S - 1))

                rinv = sp.tile([DBLK, 4], F32, name="rinv", tag="rinv")
                for hp in range(2):
                    nc.vector.reciprocal(out=rinv[:, 2 * hp:2 * hp + 2],
                                         in_=ps_o[hp][:, 256:258])
                o_sb = sp.tile([DBLK, HC], BF16 if l == 0 else F32,
                               name="o_sb", tag=f"o_sb{l}")
                for hp in range(2):
                    nc.vector.tensor_tensor(
                        out=o_sb[:, hp * 256:(hp + 1) * 256].rearrange(
                            "p (h c) -> p h c", h=2),
                        in0=ps_o[hp][:, 0:256].rearrange(
                            "p (h c) -> p h c", h=2),
                        in1=rinv[:, 2 * hp:2 * hp + 2][:, :, None]
                            .to_broadcast([DBLK, 2, 128]),
                        op=OP.mult)
                return o_sb

            def edge_B3(l, b, o_sb):
                """ELU (layer 0) and store for block b."""
                if l == 0:
                    nc.vector.tensor_mul(out=o_sb[:], in0=o_sb[:],
                                         in1=aux_b[0][2][:DBLK, :])
                    nc.vector.tensor_add(out=o_sb[:], in0=o_sb[:],
                                         in1=aux_b[0][3][:DBLK, :])
                    r_t = sp.tile([DBLK, HC], BF16, name="r_t", tag="r_t")
                    nc.scalar.activation(out=r_t[:], in_=o_sb[:],
                                         func=ACT.Relu)
                    e_t = sp.tile([DBLK, HC], BF16, name="e_t", tag="e_t")
                    nc.scalar.activation(out=e_t[:], in_=o_sb[:],
                                         func=ACT.Exp)
                    nc.vector.tensor_scalar(
                        out=e_t[:], in0=e_t[:], scalar1=-1.0, scalar2=0.0,
                        op0=OP.add, op1=OP.min)
                    h_t = sp.tile([DBLK, HC], BF16, name="h_t", tag="h_t")
                    nc.vector.tensor_add(out=h_t[:], in0=r_t[:], in1=e_t[:])
                    nc.sync.dma_start(
                        out=h_pad[b * 128:b * 128 + DBLK, :], in_=h_t[:])
                else:
                    nc.sync.dma_start(
                        out=out_d[b * DBLK:(b + 1) * DBLK, :], in_=o_sb[:])

            # ---- schedule (software pipeline per layer) -------------------
            def emit_layer_edges(l, between=None):
                pend_B2 = None   # (b, st, xa)
                pend_B3 = None   # (b, o_sb)
                ld = edge_load(l, 0)
                for b in range(NBLK + 2):
                    st = t_lrh = None
                    if b < NBLK:
                        st = edge_mm(l, b, ld)
                        t_lrh = edge_evac(l, b, st)
                    if b + 1 < NBLK:
                        ld = edge_load(l, b + 1)
                    if pend_B3 is not None:
                        edge_B3(l, *pend_B3)
                        if between is not None:
                            between(pend_B3[0])
                        pend_B3 = None
                    if st is not None:
                        xa = edge_B1(l, b, st, t_lrh)
                    if pend_B2 is not None:
                        o_sb = edge_B2(l, pend_B2[0], pend_B2[1],
                                       pend_B2[2])
                        pend_B3 = (pend_B2[0], o_sb)
                        pend_B2 = None
                    if st is not None:
                        pend_B2 = (b, st, xa)

            for g in range(10):
                phase_a_group(0, g)
                ag_chunk(0, g)

            def _between_l0(b):
                if b % 2 == 1:
                    phase_a_group(1, b // 2)
                    ag_chunk(1, b // 2)

            emit_layer_edges(0, between=_between_l0)
            emit_layer_edges(1)

    nc.compile()
    return nc


_CACHE = {}


def _get_nc(e_blk, npos_key):
    key = (e_blk, npos_key)
    if key not in _CACHE:
        _CACHE[key] = _build(e_blk, [list(npos_key[0]), list(npos_key[1])])
    return _CACHE[key]


def kernel(**inputs):
    per_core, meta = _preprocess_graph(np.asarray(inputs["edge_index"]))
    wprep, npos, col_perms, invs = _prep_weights(inputs)
    e_blk = meta["e_blk"]
    perm = meta["perm"]

    nc = _get_nc(e_blk, (tuple(npos[0]), tuple(npos[1])))

    x = np.asarray(inputs["x"], np.float32)
    x_perm = x[perm].astype(bfloat16)
    in_maps = []
    for core in range(NCORES):
        xp = np.zeros((NPAD, IN), bfloat16)
        xc = x_perm[core * NSH:(core + 1) * NSH]
        xp.reshape(NBLK, 128, IN)[:, :DBLK, :] = xc.reshape(NBLK, DBLK, IN)
        m = dict(
            x_pad=xp,
            src_idx=per_core[core]["src_idx"],
            onehot=per_core[core]["onehot"],
            onehotT=per_core[core]["onehotT"],
        )
        for l in range(2):
            m[f"wl{l}"] = wprep[f"wl{l}"]
            m[f"wr{l}"] = wprep[f"wr{l}"]
            m[f"aux{l}"] = wprep[f"aux{l}"]
        in_maps.append(m)

    trace = bool(inputs.pop("_trace", False))
    res = run_bass_kernel_spmd(nc, in_maps, core_ids=list(range(NCORES)),
                               trace=trace)
    out_rows = np.concatenate([res.results[c]["out"] for c in range(NCORES)],
                              axis=0)
    tmp = np.zeros((N, HC), np.float32)
    tmp[perm] = out_rows
    out = np.zeros((N, HC), np.float32)
    bias1 = np.asarray(inputs["bias1"], np.float32)
    out[:, col_perms[1]] = tmp * invs[1][None, :] + bias1[col_perms[1]][None, :]
    if trace:
        kernel._last_result = res
    return out


# revision 18
# speedup vs baseline: 1.1582x; 1.1582x over previous
"""Trainium2 Bass kernel for a 2-layer GATv2 encoder (nn_CG_GNN_Encoder).

kernel(**inputs) takes full inputs (x [20000,512] f32, edge_index [2,320000]
int64, weights) and returns the full [20000, 512] f32 output, across 8 cores.

v3 design (per core, dst-node sharded):
  - Host: balance dst nodes into 8 cores x 20 blocks x 125 nodes; per-block
    edge lists padded to e_blk; one-hot scatter matrices in BOTH orientations
    (edge-major `oh` for value aggregation, dst-major `ohT` for broadcasting
    dst features to edges); |att| magnitudes folded into Wl/Wr columns with
    pos-att columns ordered before neg-att per head.
  - Phase A per layer: x chunks DMA-transposed, 8 matmuls per 125-node tile
    -> xl/xr [., 512], bias added during PSUM evacuation; xl stored to DRAM
    and AllGathered in 4 chunks (overlapping phase A); xr stays local.
  - Edge phase per block: ONE batched indirect gather (xl[src]) split across
    2 SWDGE queues; xr[dst] broadcast on the tensor engine (ohT matmul) with
    xl accumulated via identity matmul; LeakyReLU applied by the scalar
    engine during PSUM->SBUF evacuation.  Per-head logits = pos-column sum
    minus neg-column sum (DVE reduces), p = exp(logit); p duplicated into
    adjacent column pairs so the value multiply runs in the DVE packed 2x
    mode; one-hot matmuls accumulate values + denominators in PSUM;
    normalize, ELU between layers.  Layer-1 output unscale/bias on host.
"""

import numpy as np
from ml_dtypes import bfloat16

import concourse.bacc as bacc
import concourse.bass as bass
import concourse.mybir as mybir
import concourse.tile as tile
from concourse.bass_utils import run_bass_kernel_spmd

F32 = mybir.dt.float32
BF16 = mybir.dt.bfloat16
I16 = mybir.dt.int16
AX = mybir.AxisListType
OP = mybir.AluOpType
ACT = mybir.ActivationFunctionType

N = 20000
H = 4
C = 128
IN = 512
HC = H * C            # 512
NEG = 0.2
NCORES = 8
NSH = N // NCORES     # 2500
DBLK = 125
NBLK = NSH // DBLK    # 20
NPAD = NBLK * 128     # 2560 padded rows (tile t at rows 128t..128t+124)
AGCH = 625            # AllGather chunk rows per core (4 chunks per layer)
ATT_EPS = 1e-10


# ----------------------------------------------------------------------------
# Host-side preprocessing
# ----------------------------------------------------------------------------

def _preprocess_graph(edge_index):
    src = np.concatenate([edge_index[0], np.arange(N, dtype=np.int64)])
    dst = np.concatenate([edge_index[1], np.arange(N, dtype=np.int64)])
    deg = np.bincount(dst, minlength=N)

    nbins = NCORES * NBLK
    order = np.argsort(-deg, kind="stable")
    import heapq
    bin_load = np.zeros(nbins, np.int64)
    bin_fill = np.zeros(nbins, np.int64)
    assign = np.zeros(N, np.int64)
    heap = [(0, b) for b in range(nbins)]
    heapq.heapify(heap)
    for nid in order:
        while True:
            load, b = heapq.heappop(heap)
            if bin_fill[b] < DBLK:
                break
        assign[nid] = b
        bin_fill[b] += 1
        bin_load[b] = load + deg[nid]
        if bin_fill[b] < DBLK:
            heapq.heappush(heap, (bin_load[b], b))

    perm = np.argsort(assign * N + np.arange(N), kind="stable")
    inv_perm = np.empty(N, np.int64)
    inv_perm[perm] = np.arange(N)

    e_bin = assign[dst]
    e_dst_pos = inv_perm[dst]
    e_src_pos = inv_perm[src]
    max_per_bin = int(np.bincount(e_bin, minlength=nbins).max())
    e_blk = -(-max_per_bin // 128) * 128
    S = e_blk // 128

    order_e = np.argsort(e_bin, kind="stable")
    eb = e_bin[order_e]
    starts = np.searchsorted(eb, np.arange(nbins))
    ends = np.searchsorted(eb, np.arange(nbins), side="right")

    E16 = e_blk // 16
    src16 = np.zeros((NCORES, NBLK, 16, E16), np.int16)
    onehot = np.zeros((NCORES, 128, NBLK, S, DBLK), bfloat16)
    onehotT = np.zeros((NCORES, DBLK, NBLK, S, 128), bfloat16)

    for b in range(nbins):
        core, blk = divmod(b, NBLK)
        sel = order_e[starts[b]:ends[b]]
        n = len(sel)
        pos = np.arange(n)
        d_loc = e_dst_pos[sel] % DBLK
        # xl_full is chunk-major: AllGather chunk c (625 rows per core) is
        # contiguous as [8 cores, 625].  Map src position -> xl_full row.
        sp_ = e_src_pos[sel]
        s_core, s_r = sp_ // NSH, sp_ % NSH
        src_row = (s_r // AGCH) * (NCORES * AGCH) + s_core * AGCH + (s_r % AGCH)
        src16[core, blk, pos % 16, pos // 16] = src_row
        onehot[core, pos % 128, blk, pos // 128, d_loc] = 1.0
        onehotT[core, d_loc, blk, pos // 128, pos % 128] = 1.0

    per_core = []
    for core in range(NCORES):
        s16 = src16[core].transpose(1, 0, 2).reshape(16, -1)
        per_core.append(dict(
            src_idx=np.tile(s16, (8, 1)).copy(),
            onehot=onehot[core].reshape(128, -1).copy(),
            onehotT=onehotT[core].reshape(DBLK, -1).copy(),
        ))
    return per_core, dict(e_blk=e_blk, perm=perm)


def _prep_weights(inputs):
    """Per layer: permute columns pos-att-first per head, scale columns by
    max(|att|, eps).  logit = sum_pos lrelu(col) - sum_neg lrelu(col)."""
    out = {}
    npos = []
    col_perms = []
    invs = []
    for l in range(2):
        att = np.asarray(inputs[f"att{l}"], np.float32)
        cols = []
        np_l = []
        for h in range(H):
            pos = np.where(att[h] >= 0)[0]
            neg = np.where(att[h] < 0)[0]
            cols.append(h * C + np.concatenate([pos, neg]))
            np_l.append(len(pos))
        cols = np.concatenate(cols)
        absa = np.maximum(np.abs(att.reshape(HC)[cols]), ATT_EPS)
        col_perms.append(cols)
        npos.append(np_l)
        invs.append((1.0 / absa).astype(np.float32))

        Wl = np.asarray(inputs[f"Wl{l}"], np.float32)
        Wr = np.asarray(inputs[f"Wr{l}"], np.float32)
        bl = np.asarray(inputs[f"bl{l}"], np.float32)
        br = np.asarray(inputs[f"br{l}"], np.float32)
        if l == 1:
            Wl = Wl[col_perms[0], :]
            Wr = Wr[col_perms[0], :]
        out[f"wl{l}"] = (Wl[:, cols] * absa[None, :]).astype(bfloat16)
        out[f"wr{l}"] = (Wr[:, cols] * absa[None, :]).astype(bfloat16)
        aux = np.zeros((4, HC), np.float32)
        aux[0] = bl[cols] * absa
        aux[1] = br[cols] * absa
        aux[2] = invs[l]
        aux[3] = np.asarray(inputs[f"bias{l}"], np.float32)[cols]
        out[f"aux{l}"] = aux.astype(bfloat16)
    return out, npos, col_perms, invs


# ----------------------------------------------------------------------------
# Device kernel
# ----------------------------------------------------------------------------

def _build(e_blk, npos):
    S = e_blk // 128
    E16 = e_blk // 16
    S0 = (S + 1) // 2          # chunks handled by queue 0
    n0 = S0 * 128
    n1 = e_blk - n0
    nc = bacc.Bacc("TRN2", target_bir_lowering=False, debug=False,
                   num_devices=NCORES, num_swdge_queues=2)

    x_in = nc.dram_tensor("x_pad", [NPAD, IN], BF16, kind="ExternalInput")
    wl_d = [nc.dram_tensor(f"wl{l}", [IN, HC], BF16, kind="ExternalInput")
            for l in range(2)]
    wr_d = [nc.dram_tensor(f"wr{l}", [IN, HC], BF16, kind="ExternalInput")
            for l in range(2)]
    aux_d = [nc.dram_tensor(f"aux{l}", [4, HC], BF16, kind="ExternalInput")
             for l in range(2)]
    srcidx_d = nc.dram_tensor("src_idx", [128, NBLK * E16], I16,
                              kind="ExternalInput")
    oh_d = nc.dram_tensor("onehot", [128, NBLK * S * DBLK], BF16,
                          kind="ExternalInput")
    ohT_d = nc.dram_tensor("onehotT", [DBLK, NBLK * S * 128], BF16,
                           kind="ExternalInput")
    out_d = nc.dram_tensor("out", [NSH, HC], F32, kind="ExternalOutput")

    from concourse.masks import make_identity

    with tile.TileContext(nc) as tc:
        with tc.tile_pool(name="dram", bufs=1, space="DRAM") as dram, \
             tc.tile_pool(name="const", bufs=1) as cp, \
             tc.tile_pool(name="pha", bufs=3) as wp, \
             tc.tile_pool(name="gath", bufs=2) as gp, \
             tc.tile_pool(name="sm", bufs=2) as sp, \
             tc.tile_pool(name="psum", bufs=2, space="PSUM") as pp:

            xl_sh = [dram.tile([NSH, HC], BF16, name=f"xl_sh{l}")
                     for l in range(2)]
            xr_dr = [dram.tile([NSH, HC], BF16, name=f"xr_dr{l}")
                     for l in range(2)]
            xl_full = [dram.tile([N, HC], BF16, name=f"xl_full{l}")
                       for l in range(2)]
            h_pad = dram.tile([NPAD, HC], BF16, name="h_pad")

            ident = cp.tile([128, 128], BF16, name="ident")
            make_identity(nc, ident[:])

            si_t = cp.tile([128, NBLK * E16], I16, name="si_t")
            nc.scalar.dma_start(out=si_t[:], in_=srcidx_d[:])

            # weights + aux broadcast tiles (both layers, resident)
            wl_t, wr_t, aux_b = [], [], []
            for l in range(2):
                wlt = cp.tile([128, 4, HC], BF16, name=f"wl_t{l}")
                wrt = cp.tile([128, 4, HC], BF16, name=f"wr_t{l}")
                for k in range(4):
                    nc.scalar.dma_start(out=wlt[:, k, :],
                                        in_=wl_d[l][k * 128:(k + 1) * 128, :])
                    nc.scalar.dma_start(out=wrt[:, k, :],
                                        in_=wr_d[l][k * 128:(k + 1) * 128, :])
                wl_t.append(wlt)
                wr_t.append(wrt)
                rows = []
                for r in range(4):
                    if l == 1 and r >= 2:
                        rows.append(None)
                        continue
                    row = cp.tile([1, HC], BF16, name=f"ar{l}{r}")
                    nc.scalar.dma_start(out=row[:], in_=aux_d[l][r:r + 1, :])
                    bc = cp.tile([128, HC], BF16, name=f"ab{l}{r}")
                    nc.gpsimd.partition_broadcast(bc[:], row[:])
                    rows.append(bc)
                aux_b.append(rows)

            # ---- emit helpers ---------------------------------------------
            def phase_a_group(l, g):
                """Tiles 5g..5g+4 (640 padded rows) of phase A for layer l."""
                src_pad = x_in if l == 0 else h_pad
                xT = wp.tile([128, 4, 640], BF16, name="xT", tag="xT")
                for k in range(4):
                    nc.sync.dma_start_transpose(
                        out=xT[:, k, :],
                        in_=src_pad[g * 640:(g + 1) * 640,
                                    k * 128:(k + 1) * 128])
                for t in range(5 * g, 5 * g + 5):
                    off = (t - 5 * g) * 128
                    ps_xl = pp.tile([DBLK, HC], F32, name="ps_xl",
                                    tag="ps_sh", bufs=2)
                    for k in range(4):
                        nc.tensor.matmul(
                            out=ps_xl[:], lhsT=xT[:, k, off:off + DBLK],
                            rhs=wl_t[l][:, k, :],
                            start=(k == 0), stop=(k == 3))
                    xl_o = wp.tile([DBLK, HC], BF16, name="xl_o", tag="xl_o")
                    nc.vector.tensor_add(out=xl_o[:], in0=ps_xl[:],
                                         in1=aux_b[l][0][:DBLK, :])
                    rows = slice(t * DBLK, (t + 1) * DBLK)
                    nc.sync.dma_start(out=xl_sh[l][rows, :], in_=xl_o[:])
                    ps_xr = pp.tile([DBLK, HC], F32, name="ps_xr",
                                    tag="ps_sh", bufs=2)
                    for k in range(4):
                        nc.tensor.matmul(
                            out=ps_xr[:], lhsT=xT[:, k, off:off + DBLK],
                            rhs=wr_t[l][:, k, :],
                            start=(k == 0), stop=(k == 3))
                    xr_o = wp.tile([DBLK, HC], BF16, name="xr_o", tag="xr_o")
                    nc.vector.tensor_add(out=xr_o[:], in0=ps_xr[:],
                                         in1=aux_b[l][1][:DBLK, :])
                    nc.sync.dma_start(out=xr_dr[l][rows, :], in_=xr_o[:])

            def ag_chunk(l, c):
                nc.gpsimd.collective_compute(
                    "AllGather", OP.bypass,
                    replica_groups=[list(range(NCORES))],
                    ins=[xl_sh[l][c * AGCH:(c + 1) * AGCH, :]],
                    outs=[xl_full[l][c * NCORES * AGCH:
                                     (c + 1) * NCORES * AGCH, :]],
                )

            # halves: half 0 = s in [0, S0), half 1 = s in [S0, S)
            halves = [(0, S0), (S0, S)]

            def edge_load(l, b):
                """Gathers + streams for block b (both halves)."""
                io0 = b * E16
                xl_gh = []
                for hf, (sa, sb_) in enumerate(halves):
                    nh = (sb_ - sa) * 128
                    xg = gp.tile([128, S0, HC], BF16, name=f"xl_g{hf}",
                                 tag=f"xl_g{hf}")
                    nc.gpsimd.dma_gather(
                        out_ap=xg[:, 0:sb_ - sa, :], in_ap=xl_full[l][:],
                        idxs_ap=si_t[:, io0 + sa * 8:io0 + sa * 8 + nh // 16],
                        num_idxs=nh, num_idxs_reg=nh, elem_size=HC,
                        single_packet=False, queue_num=hf)
                    xl_gh.append(xg)

                oh_b = gp.tile([128, S, DBLK], BF16, name="oh_b", tag="oh_b")
                nc.sync.dma_start(
                    out=oh_b[:],
                    in_=oh_d[:, b * S * DBLK:(b + 1) * S * DBLK])
                ohT_b = gp.tile([DBLK, S, 128], BF16, name="ohT_b",
                                tag="ohT_b")
                nc.sync.dma_start(
                    out=ohT_b[:],
                    in_=ohT_d[:, b * S * 128:(b + 1) * S * 128])
                xr_b = gp.tile([DBLK, HC], BF16, name="xr_b", tag="xr_b")
                nc.sync.dma_start(
                    out=xr_b[:], in_=xr_dr[l][b * DBLK:(b + 1) * DBLK, :])
                return xl_gh, oh_b, ohT_b, xr_b

            def edge_mm(l, b, ld):
                """t = xr[dst] + xl[src] matmuls into PSUM."""
                xl_gh, oh_b, ohT_b, xr_b = ld
                ps_lists = [[], []]
                for hf, (sa, sb_) in enumerate(halves):
                    for s2 in range(sa, sb_, 2):
                        w = min(2, sb_ - s2)
                        ps_t = pp.tile([128, 2, HC], F32, name="ps_t",
                                       tag="ps_t", bufs=3)
                        for s in range(s2, s2 + w):
                            nc.tensor.matmul(out=ps_t[:, s - s2, :],
                                             lhsT=ohT_b[:, s, :], rhs=xr_b[:],
                                             start=True, stop=False)
                            nc.tensor.matmul(
                                out=ps_t[:, s - s2, :], lhsT=ident[:],
                                rhs=xl_gh[hf][:, s - sa, :],
                                start=False, stop=True)
                        ps_lists[hf].append((s2, w, ps_t))
                return xl_gh, oh_b, ps_lists

            def edge_evac(l, b, st):
                """ACT LeakyReLU evacuation PSUM -> bf16 SBUF, per half."""
                xl_gh, oh_b, ps_lists = st
                t_lrh = []
                for hf, (sa, sb_) in enumerate(halves):
                    t_lr = gp.tile([128, S0, HC], BF16, name=f"t_lr{hf}",
                                   tag=f"t_lr{hf}")
                    for s2, w, ps_t in ps_lists[hf]:
                        nc.scalar.activation(
                            out=t_lr[:, s2 - sa:s2 - sa + w, :],
                            in_=ps_t[:, 0:w, :], func=ACT.Prelu, alpha=NEG)
                    t_lrh.append(t_lr)
                return t_lrh

            def edge_B1(l, b, st, t_lrh):
                """Per half: logit reduces + exp + value multiply."""
                xl_gh, oh_b, ps_lists = st
                xa = gp.tile([128, S, 2, 258], BF16, name="xa", tag="xa")
                for hf, (sa, sb_) in enumerate(halves):
                    ns = sb_ - sa
                    t_lr = t_lrh[hf]
                    lg_pn = sp.tile([128, 2, S0, H], F32, name=f"lg_pn{hf}",
                                    tag=f"lg_pn{hf}")
                    for h in range(H):
                        np_h = npos[l][h]
                        lo, mid, hi = h * C, h * C + np_h, (h + 1) * C
                        if np_h > 0:
                            nc.vector.tensor_reduce(
                                out=lg_pn[:, 0, 0:ns, h],
                                in_=t_lr[:, 0:ns, lo:mid],
                                axis=AX.X, op=OP.add)
                        else:
                            nc.vector.memset(lg_pn[:, 0, 0:ns, h], 0.0)
                        if np_h < C:
                            nc.vector.tensor_reduce(
                                out=lg_pn[:, 1, 0:ns, h],
                                in_=t_lr[:, 0:ns, mid:hi],
                                axis=AX.X, op=OP.add)
                        else:
                            nc.vector.memset(lg_pn[:, 1, 0:ns, h], 0.0)
                    lg = sp.tile([128, S0, H], F32, name=f"lg{hf}",
                                 tag=f"lg{hf}")
                    nc.vector.tensor_tensor(out=lg[:, 0:ns, :],
                                            in0=lg_pn[:, 0, 0:ns, :],
                                            in1=lg_pn[:, 1, 0:ns, :],
                                            op=OP.subtract)
                    # p = exp(lg): denominator cols + duplicated pairs
                    nc.scalar.activation(
                        out=xa[:, sa:sb_, :, 256:258],
                        in_=lg[:, 0:ns, :].rearrange(
                            "p s (a b) -> p s a b", a=2),
                        func=ACT.Exp)
                    p_dup = sp.tile([128, S0, H, 2], BF16, name=f"p_dup{hf}",
                                    tag=f"p_dup{hf}")
                    nc.scalar.activation(out=p_dup[:, 0:ns, :, 0],
                                         in_=lg[:, 0:ns, :], func=ACT.Exp)
                    nc.scalar.activation(out=p_dup[:, 0:ns, :, 1],
                                         in_=lg[:, 0:ns, :], func=ACT.Exp)
                    # xa = xl * p (packed 2x)
                    for h in range(H):
                        hp, hh = divmod(h, 2)
                        nc.vector.tensor_tensor(
                            out=xa[:, sa:sb_, hp,
                                   hh * 128:(hh + 1) * 128].rearrange(
                                "p s (pr two) -> p s pr two", two=2),
                            in0=xl_gh[hf][:, 0:ns,
                                          h * 128:(h + 1) * 128].rearrange(
                                "p s (pr two) -> p s pr two", two=2),
                            in1=p_dup[:, 0:ns, h, None, :]
                                .to_broadcast([128, ns, 64, 2]),
                            op=OP.mult)
                return xa

            def edge_B2(l, b, st, xa):
                """Value one-hot matmuls + normalize; ELU/store deferred."""
                xl_gh, oh_b, ps_lists = st
                ps_o = [pp.tile([DBLK, HC], F32, name=f"ps_o{hp}",
                                tag="ps_sh", bufs=2) for hp in range(2)]
                for s in range(S):
                    for hp in range(2):
                        nc.tensor.matmul(out=ps_o[hp][:, 0:258],
                                         lhsT=oh_b[:, s, :],
                                         rhs=xa[:, s, hp, :],
                                         start=(s == 0), stop=(s == S - 1))

                rinv = sp.tile([DBLK, 4], F32, name="rinv", tag="rinv")
                for hp in range(2):
                    nc.vector.reciprocal(out=rinv[:, 2 * hp:2 * hp + 2],
                                         in_=ps_o[hp][:, 256:258])
                o_sb = sp.tile([DBLK, HC], BF16 if l == 0 else F32,
                               name="o_sb", tag=f"o_sb{l}")
                for hp in range(2):
                    nc.vector.tensor_tensor(
                        out=o_sb[:, hp * 256:(hp + 1) * 256].rearrange(
                            "p (h c) -> p h c", h=2),
                        in0=ps_o[hp][:, 0:256].rearrange(
                            "p (h c) -> p h c", h=2),
                        in1=rinv[:, 2 * hp:2 * hp + 2][:, :, None]
                            .to_broadcast([DBLK, 2, 128]),
                        op=OP.mult)
                return o_sb

            def edge_B3(l, b, o_sb):
                """ELU (layer 0) and store for block b."""
                if l == 0:
                    nc.vector.tensor_mul(out=o_sb[:], in0=o_sb[:],
                                         in1=aux_b[0][2][:DBLK, :])
                    nc.vector.tensor_add(out=o_sb[:], in0=o_sb[:],
                                         in1=aux_b[0][3][:DBLK, :])
                    r_t = sp.tile([DBLK, HC], BF16, name="r_t", tag="r_t")
                    nc.scalar.activation(out=r_t[:], in_=o_sb[:],
                                         func=ACT.Relu)
                    e_t = sp.tile([DBLK, HC], BF16, name="e_t", tag="e_t")
                    nc.scalar.activation(out=e_t[:], in_=o_sb[:],
                                         func=ACT.Exp)
                    nc.vector.tensor_scalar(
                        out=e_t[:], in0=e_t[:], scalar1=-1.0, scalar2=0.0,
                        op0=OP.add, op1=OP.min)
                    h_t = sp.tile([DBLK, HC], BF16, name="h_t", tag="h_t")
                    nc.vector.tensor_add(out=h_t[:], in0=r_t[:], in1=e_t[:])
                    nc.sync.dma_start(
                        out=h_pad[b * 128:b * 128 + DBLK, :], in_=h_t[:])
                else:
                    nc.sync.dma_start(
                        out=out_d[b * DBLK:(b + 1) * DBLK, :], in_=o_sb[:])

            # ---- schedule (software pipeline per layer) -------------------
            def emit_layer_edges(l, between=None):
                pend_B2 = None   # (b, st, xa)
                pend_B3 = None   # (b, o_sb)
                ld = edge_load(l, 0)
                for b in range(NBLK + 2):
                    st = t_lrh = None
                    if b < NBLK:
                        st = edge_mm(l, b, ld)
                        t_lrh = edge_evac(l, b, st)
                    if b + 1 < NBLK:
                        ld = edge_load(l, b + 1)
                    if pend_B3 is not None:
                        edge_B3(l, *pend_B3)
                        if between is not None:
                            between(pend_B3[0])
                        pend_B3 = None
                    if st is not None:
                        xa = edge_B1(l, b, st, t_lrh)
                    if pend_B2 is not None:
                        o_sb = edge_B2(l, pend_B2[0], pend_B2[1],
                                       pend_B2[2])
                        pend_B3 = (pend_B2[0], o_sb)
                        pend_B2 = None
                    if st is not None:
                        pend_B2 = (b, st, xa)

            for g in range(4):
                phase_a_group(0, g)
                ag_chunk(0, g)

            def _between_l0(b):
                if b % 5 == 4:
                    phase_a_group(1, b // 5)
                    ag_chunk(1, b // 5)

            emit_layer_edges(0, between=_between_l0)
            emit_layer_edges(1)

    nc.compile()
    return nc


_CACHE = {}


def _get_nc(e_blk, npos_key):
    key = (e_blk, npos_key)
    if key not in _CACHE:
        _CACHE[key] = _build(e_blk, [list(npos_key[0]), list(npos_key[1])])
    return _CACHE[key]


def kernel(**inputs):
    per_core, meta = _preprocess_graph(np.asarray(inputs["edge_index"]))
    wprep, npos, col_perms, invs = _prep_weights(inputs)
    e_blk = meta["e_blk"]
    perm = meta["perm"]

    nc = _get_nc(e_blk, (tuple(npos[0]), tuple(npos[1])))

    x = np.asarray(inputs["x"], np.float32)
    x_perm = x[perm].astype(bfloat16)
    in_maps = []
    for core in range(NCORES):
        xp = np.zeros((NPAD, IN), bfloat16)
        xc = x_perm[core * NSH:(core + 1) * NSH]
        xp.reshape(NBLK, 128, IN)[:, :DBLK, :] = xc.reshape(NBLK, DBLK, IN)
        m = dict(
            x_pad=xp,
            src_idx=per_core[core]["src_idx"],
            onehot=per_core[core]["onehot"],
            onehotT=per_core[core]["onehotT"],
        )
        for l in range(2):
            m[f"wl{l}"] = wprep[f"wl{l}"]
            m[f"wr{l}"] = wprep[f"wr{l}"]
            m[f"aux{l}"] = wprep[f"aux{l}"]
        in_maps.append(m)

    trace = bool(inputs.pop("_trace", False))
    res = run_bass_kernel_spmd(nc, in_maps, core_ids=list(range(NCORES)),
                               trace=trace)
    out_rows = np.concatenate([res.results[c]["out"] for c in range(NCORES)],
                              axis=0)
    tmp = np.zeros((N, HC), np.float32)
    tmp[perm] = out_rows
    out = np.zeros((N, HC), np.float32)
    bias1 = np.asarray(inputs["bias1"], np.float32)
    out[:, col_perms[1]] = tmp * invs[1][None, :] + bias1[col_perms[1]][None, :]
    if trace:
        kernel._last_result = res
    return out


# revision 19
# speedup vs baseline: 1.2299x; 1.0619x over previous
"""Trainium2 Bass kernel for a 2-layer GATv2 encoder (nn_CG_GNN_Encoder).

kernel(**inputs) takes full inputs (x [20000,512] f32, edge_index [2,320000]
int64, weights) and returns the full [20000, 512] f32 output, across 8 cores.

v3 design (per core, dst-node sharded):
  - Host: balance dst nodes into 8 cores x 20 blocks x 125 nodes; per-block
    edge lists padded to e_blk; one-hot scatter matrices in BOTH orientations
    (edge-major `oh` for value aggregation, dst-major `ohT` for broadcasting
    dst features to edges); |att| magnitudes folded into Wl/Wr columns with
    pos-att columns ordered before neg-att per head.
  - Phase A per layer: x chunks DMA-transposed, 8 matmuls per 125-node tile
    -> xl/xr [., 512], bias added during PSUM evacuation; xl stored to DRAM
    and AllGathered in 4 chunks (overlapping phase A); xr stays local.
  - Edge phase per block: ONE batched indirect gather (xl[src]) split across
    2 SWDGE queues; xr[dst] broadcast on the tensor engine (ohT matmul) with
    xl accumulated via identity matmul; LeakyReLU applied by the scalar
    engine during PSUM->SBUF evacuation.  Per-head logits = pos-column sum
    minus neg-column sum (DVE reduces), p = exp(logit); p duplicated into
    adjacent column pairs so the value multiply runs in the DVE packed 2x
    mode; one-hot matmuls accumulate values + denominators in PSUM;
    normalize, ELU between layers.  Layer-1 output unscale/bias on host.
"""

import numpy as np
from ml_dtypes import bfloat16

import concourse.bacc as bacc
import concourse.bass as bass
import concourse.mybir as mybir
import concourse.tile as tile
from concourse.bass_utils import run_bass_kernel_spmd

F32 = mybir.dt.float32
BF16 = mybir.dt.bfloat16
I16 = mybir.dt.int16
AX = mybir.AxisListType
OP = mybir.AluOpType
ACT = mybir.ActivationFunctionType

N = 20000
H = 4
C = 128
IN = 512
HC = H * C            # 512
NEG = 0.2
NCORES = 8
NSH = N // NCORES     # 2500
DBLK = 125
NBLK = NSH // DBLK    # 20
NPAD = NBLK * 128     # 2560 padded rows (tile t at rows 128t..128t+124)
AGCH = 625            # AllGather chunk rows per core (4 chunks per layer)
ATT_EPS = 1e-10


# ----------------------------------------------------------------------------
# Host-side preprocessing
# ----------------------------------------------------------------------------

def _preprocess_graph(edge_index):
    src = np.concatenate([edge_index[0], np.arange(N, dtype=np.int64)])
    dst = np.concatenate([edge_index[1], np.arange(N, dtype=np.int64)])
    deg = np.bincount(dst, minlength=N)

    nbins = NCORES * NBLK
    order = np.argsort(-deg, kind="stable")
    import heapq
    bin_load = np.zeros(nbins, np.int64)
    bin_fill = np.zeros(nbins, np.int64)
    assign = np.zeros(N, np.int64)
    heap = [(0, b) for b in range(nbins)]
    heapq.heapify(heap)
    for nid in order:
        while True:
            load, b = heapq.heappop(heap)
            if bin_fill[b] < DBLK:
                break
        assign[nid] = b
        bin_fill[b] += 1
        bin_load[b] = load + deg[nid]
        if bin_fill[b] < DBLK:
            heapq.heappush(heap, (bin_load[b], b))

    perm = np.argsort(assign * N + np.arange(N), kind="stable")
    inv_perm = np.empty(N, np.int64)
    inv_perm[perm] = np.arange(N)

    e_bin = assign[dst]
    e_dst_pos = inv_perm[dst]
    e_src_pos = inv_perm[src]
    max_per_bin = int(np.bincount(e_bin, minlength=nbins).max())
    e_blk = -(-max_per_bin // 128) * 128
    S = e_blk // 128

    order_e = np.argsort(e_bin, kind="stable")
    eb = e_bin[order_e]
    starts = np.searchsorted(eb, np.arange(nbins))
    ends = np.searchsorted(eb, np.arange(nbins), side="right")

    E16 = e_blk // 16
    src16 = np.zeros((NCORES, NBLK, 16, E16), np.int16)
    onehot = np.zeros((NCORES, 128, NBLK, S, DBLK), bfloat16)
    onehotT = np.zeros((NCORES, DBLK, NBLK, S, 128), bfloat16)

    for b in range(nbins):
        core, blk = divmod(b, NBLK)
        sel = order_e[starts[b]:ends[b]]
        n = len(sel)
        pos = np.arange(n)
        d_loc = e_dst_pos[sel] % DBLK
        # xl_full is chunk-major: AllGather chunk c (625 rows per core) is
        # contiguous as [8 cores, 625].  Map src position -> xl_full row.
        sp_ = e_src_pos[sel]
        s_core, s_r = sp_ // NSH, sp_ % NSH
        src_row = (s_r // AGCH) * (NCORES * AGCH) + s_core * AGCH + (s_r % AGCH)
        src16[core, blk, pos % 16, pos // 16] = src_row
        onehot[core, pos % 128, blk, pos // 128, d_loc] = 1.0
        onehotT[core, d_loc, blk, pos // 128, pos % 128] = 1.0

    per_core = []
    for core in range(NCORES):
        s16 = src16[core].transpose(1, 0, 2).reshape(16, -1)
        per_core.append(dict(
            src_idx=np.tile(s16, (8, 1)).copy(),
            onehot=onehot[core].reshape(128, -1).copy(),
            onehotT=onehotT[core].reshape(DBLK, -1).copy(),
        ))
    return per_core, dict(e_blk=e_blk, perm=perm)


def _prep_weights(inputs):
    """Per layer: permute columns pos-att-first per head, scale columns by
    max(|att|, eps).  logit = sum_pos lrelu(col) - sum_neg lrelu(col)."""
    out = {}
    npos = []
    col_perms = []
    invs = []
    for l in range(2):
        att = np.asarray(inputs[f"att{l}"], np.float32)
        cols = []
        np_l = []
        for h in range(H):
            pos = np.where(att[h] >= 0)[0]
            neg = np.where(att[h] < 0)[0]
            cols.append(h * C + np.concatenate([pos, neg]))
            np_l.append(len(pos))
        cols = np.concatenate(cols)
        absa = np.maximum(np.abs(att.reshape(HC)[cols]), ATT_EPS)
        col_perms.append(cols)
        npos.append(np_l)
        invs.append((1.0 / absa).astype(np.float32))

        Wl = np.asarray(inputs[f"Wl{l}"], np.float32)
        Wr = np.asarray(inputs[f"Wr{l}"], np.float32)
        bl = np.asarray(inputs[f"bl{l}"], np.float32)
        br = np.asarray(inputs[f"br{l}"], np.float32)
        if l == 1:
            Wl = Wl[col_perms[0], :]
            Wr = Wr[col_perms[0], :]
        out[f"wl{l}"] = (Wl[:, cols] * absa[None, :]).astype(bfloat16)
        out[f"wr{l}"] = (Wr[:, cols] * absa[None, :]).astype(bfloat16)
        aux = np.zeros((4, HC), np.float32)
        aux[0] = bl[cols] * absa
        aux[1] = br[cols] * absa
        aux[2] = invs[l]
        aux[3] = np.asarray(inputs[f"bias{l}"], np.float32)[cols]
        out[f"aux{l}"] = aux.astype(bfloat16)
    return out, npos, col_perms, invs


# ----------------------------------------------------------------------------
# Device kernel
# ----------------------------------------------------------------------------

def _build(e_blk, npos):
    S = e_blk // 128
    E16 = e_blk // 16
    S0 = (S + 1) // 2          # chunks handled by queue 0
    n0 = S0 * 128
    n1 = e_blk - n0
    nc = bacc.Bacc("TRN2", target_bir_lowering=False, debug=False,
                   num_devices=NCORES, num_swdge_queues=2)

    x_in = nc.dram_tensor("x_pad", [NPAD, IN], BF16, kind="ExternalInput")
    wl_d = [nc.dram_tensor(f"wl{l}", [IN, HC], BF16, kind="ExternalInput")
            for l in range(2)]
    wr_d = [nc.dram_tensor(f"wr{l}", [IN, HC], BF16, kind="ExternalInput")
            for l in range(2)]
    aux_d = [nc.dram_tensor(f"aux{l}", [4, HC], BF16, kind="ExternalInput")
             for l in range(2)]
    srcidx_d = nc.dram_tensor("src_idx", [128, NBLK * E16], I16,
                              kind="ExternalInput")
    oh_d = nc.dram_tensor("onehot", [128, NBLK * S * DBLK], BF16,
                          kind="ExternalInput")
    ohT_d = nc.dram_tensor("onehotT", [DBLK, NBLK * S * 128], BF16,
                           kind="ExternalInput")
    out_d = nc.dram_tensor("out", [NSH, HC], F32, kind="ExternalOutput")

    from concourse.masks import make_identity

    with tile.TileContext(nc) as tc:
        with tc.tile_pool(name="dram", bufs=1, space="DRAM") as dram, \
             tc.tile_pool(name="const", bufs=1) as cp, \
             tc.tile_pool(name="pha", bufs=2) as wp, \
             tc.tile_pool(name="gath", bufs=2) as gp, \
             tc.tile_pool(name="sm", bufs=2) as sp, \
             tc.tile_pool(name="psum", bufs=2, space="PSUM") as pp:

            xl_sh = [dram.tile([NSH, HC], BF16, name=f"xl_sh{l}")
                     for l in range(2)]
            xr_dr = [dram.tile([NSH, HC], BF16, name=f"xr_dr{l}")
                     for l in range(2)]
            xl_full = [dram.tile([N, HC], BF16, name=f"xl_full{l}")
                       for l in range(2)]
            h_pad = dram.tile([NPAD, HC], BF16, name="h_pad")

            ident = cp.tile([128, 128], BF16, name="ident")
            make_identity(nc, ident[:])

            si_t = cp.tile([128, NBLK * E16], I16, name="si_t")
            nc.scalar.dma_start(out=si_t[:], in_=srcidx_d[:])

            # weights + aux broadcast tiles (both layers, resident)
            wl_t, wr_t, aux_b = [], [], []
            for l in range(2):
                wlt = cp.tile([128, 4, HC], BF16, name=f"wl_t{l}")
                wrt = cp.tile([128, 4, HC], BF16, name=f"wr_t{l}")
                for k in range(4):
                    nc.scalar.dma_start(out=wlt[:, k, :],
                                        in_=wl_d[l][k * 128:(k + 1) * 128, :])
                    nc.scalar.dma_start(out=wrt[:, k, :],
                                        in_=wr_d[l][k * 128:(k + 1) * 128, :])
                wl_t.append(wlt)
                wr_t.append(wrt)
                rows = []
                for r in range(4):
                    if l == 1 and r >= 2:
                        rows.append(None)
                        continue
                    row = cp.tile([1, HC], BF16, name=f"ar{l}{r}")
                    nc.scalar.dma_start(out=row[:], in_=aux_d[l][r:r + 1, :])
                    bc = cp.tile([128, HC], BF16, name=f"ab{l}{r}")
                    nc.gpsimd.partition_broadcast(bc[:], row[:])
                    rows.append(bc)
                aux_b.append(rows)

            # ---- emit helpers ---------------------------------------------
            def phase_a_group(l, g):
                """Tiles 5g..5g+4 (640 padded rows) of phase A for layer l."""
                src_pad = x_in if l == 0 else h_pad
                xT = wp.tile([128, 4, 640], BF16, name="xT", tag="xT")
                for k in range(4):
                    nc.sync.dma_start_transpose(
                        out=xT[:, k, :],
                        in_=src_pad[g * 640:(g + 1) * 640,
                                    k * 128:(k + 1) * 128])
                for t in range(5 * g, 5 * g + 5):
                    off = (t - 5 * g) * 128
                    ps_xl = pp.tile([DBLK, HC], F32, name="ps_xl",
                                    tag="ps_sh", bufs=2)
                    for k in range(4):
                        nc.tensor.matmul(
                            out=ps_xl[:], lhsT=xT[:, k, off:off + DBLK],
                            rhs=wl_t[l][:, k, :],
                            start=(k == 0), stop=(k == 3))
                    xl_o = wp.tile([DBLK, HC], BF16, name="xl_o", tag="xl_o")
                    nc.vector.tensor_add(out=xl_o[:], in0=ps_xl[:],
                                         in1=aux_b[l][0][:DBLK, :])
                    rows = slice(t * DBLK, (t + 1) * DBLK)
                    nc.sync.dma_start(out=xl_sh[l][rows, :], in_=xl_o[:])
                    ps_xr = pp.tile([DBLK, HC], F32, name="ps_xr",
                                    tag="ps_sh", bufs=2)
                    for k in range(4):
                        nc.tensor.matmul(
                            out=ps_xr[:], lhsT=xT[:, k, off:off + DBLK],
                            rhs=wr_t[l][:, k, :],
                            start=(k == 0), stop=(k == 3))
                    xr_o = wp.tile([DBLK, HC], BF16, name="xr_o", tag="xr_o")
                    nc.vector.tensor_add(out=xr_o[:], in0=ps_xr[:],
                                         in1=aux_b[l][1][:DBLK, :])
                    nc.sync.dma_start(out=xr_dr[l][rows, :], in_=xr_o[:])

            def ag_chunk(l, c):
                nc.gpsimd.collective_compute(
                    "AllGather", OP.bypass,
                    replica_groups=[list(range(NCORES))],
                    ins=[xl_sh[l][c * AGCH:(c + 1) * AGCH, :]],
                    outs=[xl_full[l][c * NCORES * AGCH:
                                     (c + 1) * NCORES * AGCH, :]],
                )

            # halves: half 0 = s in [0, S0), half 1 = s in [S0, S)
            halves = [(0, S0), (S0, S)]

            def edge_load(l, b):
                """Gathers + streams for block b (both halves)."""
                io0 = b * E16
                xl_gh = []
                for hf, (sa, sb_) in enumerate(halves):
                    nh = (sb_ - sa) * 128
                    xg = gp.tile([128, S0, HC], BF16, name=f"xl_g{hf}",
                                 tag=f"xl_g{hf}", bufs=3)
                    nc.gpsimd.dma_gather(
                        out_ap=xg[:, 0:sb_ - sa, :], in_ap=xl_full[l][:],
                        idxs_ap=si_t[:, io0 + sa * 8:io0 + sa * 8 + nh // 16],
                        num_idxs=nh, num_idxs_reg=nh, elem_size=HC,
                        single_packet=False, queue_num=hf)
                    xl_gh.append(xg)

                oh_b = gp.tile([128, S, DBLK], BF16, name="oh_b", tag="oh_b")
                nc.sync.dma_start(
                    out=oh_b[:],
                    in_=oh_d[:, b * S * DBLK:(b + 1) * S * DBLK])
                ohT_b = gp.tile([DBLK, S, 128], BF16, name="ohT_b",
                                tag="ohT_b")
                nc.sync.dma_start(
                    out=ohT_b[:],
                    in_=ohT_d[:, b * S * 128:(b + 1) * S * 128])
                xr_b = gp.tile([DBLK, HC], BF16, name="xr_b", tag="xr_b")
                nc.sync.dma_start(
                    out=xr_b[:], in_=xr_dr[l][b * DBLK:(b + 1) * DBLK, :])
                return xl_gh, oh_b, ohT_b, xr_b

            def edge_mm(l, b, ld):
                """t = xr[dst] + xl[src] matmuls into PSUM."""
                xl_gh, oh_b, ohT_b, xr_b = ld
                ps_lists = [[], []]
                for hf, (sa, sb_) in enumerate(halves):
                    for s2 in range(sa, sb_, 2):
                        w = min(2, sb_ - s2)
                        ps_t = pp.tile([128, 2, HC], F32, name="ps_t",
                                       tag="ps_t", bufs=3)
                        for s in range(s2, s2 + w):
                            nc.tensor.matmul(out=ps_t[:, s - s2, :],
                                             lhsT=ohT_b[:, s, :], rhs=xr_b[:],
                                             start=True, stop=False)
                            nc.tensor.matmul(
                                out=ps_t[:, s - s2, :], lhsT=ident[:],
                                rhs=xl_gh[hf][:, s - sa, :],
                                start=False, stop=True)
                        ps_lists[hf].append((s2, w, ps_t))
                return xl_gh, oh_b, ps_lists

            def edge_evac(l, b, st):
                """ACT LeakyReLU evacuation PSUM -> bf16 SBUF, per half."""
                xl_gh, oh_b, ps_lists = st
                t_lrh = []
                for hf, (sa, sb_) in enumerate(halves):
                    t_lr = gp.tile([128, S0, HC], BF16, name=f"t_lr{hf}",
                                   tag=f"t_lr{hf}")
                    for s2, w, ps_t in ps_lists[hf]:
                        nc.scalar.activation(
                            out=t_lr[:, s2 - sa:s2 - sa + w, :],
                            in_=ps_t[:, 0:w, :], func=ACT.Prelu, alpha=NEG)
                    t_lrh.append(t_lr)
                return t_lrh

            def edge_B1(l, b, st, t_lrh):
                """Per half: logit reduces + exp + value multiply."""
                xl_gh, oh_b, ps_lists = st
                xa = gp.tile([128, S, 2, 258], BF16, name="xa", tag="xa")
                for hf, (sa, sb_) in enumerate(halves):
                    ns = sb_ - sa
                    t_lr = t_lrh[hf]
                    lg_pn = sp.tile([128, 2, S0, H], F32, name=f"lg_pn{hf}",
                                    tag=f"lg_pn{hf}")
                    for h in range(H):
                        np_h = npos[l][h]
                        lo, mid, hi = h * C, h * C + np_h, (h + 1) * C
                        if np_h > 0:
                            nc.vector.tensor_reduce(
                                out=lg_pn[:, 0, 0:ns, h],
                                in_=t_lr[:, 0:ns, lo:mid],
                                axis=AX.X, op=OP.add)
                        else:
                            nc.vector.memset(lg_pn[:, 0, 0:ns, h], 0.0)
                        if np_h < C:
                            nc.vector.tensor_reduce(
                                out=lg_pn[:, 1, 0:ns, h],
                                in_=t_lr[:, 0:ns, mid:hi],
                                axis=AX.X, op=OP.add)
                        else:
                            nc.vector.memset(lg_pn[:, 1, 0:ns, h], 0.0)
                    lg = sp.tile([128, S0, H], F32, name=f"lg{hf}",
                                 tag=f"lg{hf}")
                    nc.vector.tensor_tensor(out=lg[:, 0:ns, :],
                                            in0=lg_pn[:, 0, 0:ns, :],
                                            in1=lg_pn[:, 1, 0:ns, :],
                                            op=OP.subtract)
                    # p = exp(lg): denominator cols + duplicated pairs
                    nc.scalar.activation(
                        out=xa[:, sa:sb_, :, 256:258],
                        in_=lg[:, 0:ns, :].rearrange(
                            "p s (a b) -> p s a b", a=2),
                        func=ACT.Exp)
                    p_dup = sp.tile([128, S0, H, 2], BF16, name=f"p_dup{hf}",
                                    tag=f"p_dup{hf}")
                    nc.scalar.activation(out=p_dup[:, 0:ns, :, 0],
                                         in_=lg[:, 0:ns, :], func=ACT.Exp)
                    nc.scalar.activation(out=p_dup[:, 0:ns, :, 1],
                                         in_=lg[:, 0:ns, :], func=ACT.Exp)
                    # xa = xl * p (packed 2x)
                    for h in range(H):
                        hp, hh = divmod(h, 2)
                        nc.vector.tensor_tensor(
                            out=xa[:, sa:sb_, hp,
                                   hh * 128:(hh + 1) * 128].rearrange(
                                "p s (pr two) -> p s pr two", two=2),
                            in0=xl_gh[hf][:, 0:ns,
                                          h * 128:(h + 1) * 128].rearrange(
                                "p s (pr two) -> p s pr two", two=2),
                            in1=p_dup[:, 0:ns, h, None, :]
                                .to_broadcast([128, ns, 64, 2]),
                            op=OP.mult)
                return xa

            def edge_B2(l, b, st, xa):
                """Value one-hot matmuls + normalize; ELU/store deferred."""
                xl_gh, oh_b, ps_lists = st
                ps_o = [pp.tile([DBLK, HC], F32, name=f"ps_o{hp}",
                                tag="ps_sh", bufs=2) for hp in range(2)]
                for s in range(S):
                    for hp in range(2):
                        nc.tensor.matmul(out=ps_o[hp][:, 0:258],
                                         lhsT=oh_b[:, s, :],
                                         rhs=xa[:, s, hp, :],
                                         start=(s == 0), stop=(s == S - 1))

                rinv = sp.tile([DBLK, 4], F32, name="rinv", tag="rinv")
                for hp in range(2):
                    nc.vector.reciprocal(out=rinv[:, 2 * hp:2 * hp + 2],
                                         in_=ps_o[hp][:, 256:258])
                o_sb = sp.tile([DBLK, HC], BF16 if l == 0 else F32,
                               name="o_sb", tag=f"o_sb{l}")
                for hp in range(2):
                    nc.vector.tensor_tensor(
                        out=o_sb[:, hp * 256:(hp + 1) * 256].rearrange(
                            "p (h c) -> p h c", h=2),
                        in0=ps_o[hp][:, 0:256].rearrange(
                            "p (h c) -> p h c", h=2),
                        in1=rinv[:, 2 * hp:2 * hp + 2][:, :, None]
                            .to_broadcast([DBLK, 2, 128]),
                        op=OP.mult)
                return o_sb

            def edge_B3(l, b, o_sb):
                """ELU (layer 0) and store for block b."""
                if l == 0:
                    nc.vector.tensor_mul(out=o_sb[:], in0=o_sb[:],
                                         in1=aux_b[0][2][:DBLK, :])
                    nc.vector.tensor_add(out=o_sb[:], in0=o_sb[:],
                                         in1=aux_b[0][3][:DBLK, :])
                    r_t = sp.tile([DBLK, HC], BF16, name="r_t", tag="r_t")
                    nc.scalar.activation(out=r_t[:], in_=o_sb[:],
                                         func=ACT.Relu)
                    e_t = sp.tile([DBLK, HC], BF16, name="e_t", tag="e_t")
                    nc.scalar.activation(out=e_t[:], in_=o_sb[:],
                                         func=ACT.Exp)
                    nc.vector.tensor_scalar(
                        out=e_t[:], in0=e_t[:], scalar1=-1.0, scalar2=0.0,
                        op0=OP.add, op1=OP.min)
                    h_t = sp.tile([DBLK, HC], BF16, name="h_t", tag="h_t")
                    nc.vector.tensor_add(out=h_t[:], in0=r_t[:], in1=e_t[:])
                    nc.sync.dma_start(
                        out=h_pad[b * 128:b * 128 + DBLK, :], in_=h_t[:])
                else:
                    nc.sync.dma_start(
                        out=out_d[b * DBLK:(b + 1) * DBLK, :], in_=o_sb[:])

            # ---- schedule (software pipeline per layer) -------------------
            def emit_layer_edges(l, between=None):
                pend_B2 = None   # (b, st, xa)
                pend_B3 = None   # (b, o_sb)
                ld = edge_load(l, 0)
                for b in range(NBLK + 2):
                    st = t_lrh = None
                    if b < NBLK:
                        st = edge_mm(l, b, ld)
                        t_lrh = edge_evac(l, b, st)
                    if b + 1 < NBLK:
                        ld = edge_load(l, b + 1)
                    if pend_B3 is not None:
                        edge_B3(l, *pend_B3)
                        if between is not None:
                            between(pend_B3[0])
                        pend_B3 = None
                    if st is not None:
                        xa = edge_B1(l, b, st, t_lrh)
                    if pend_B2 is not None:
                        o_sb = edge_B2(l, pend_B2[0], pend_B2[1],
                                       pend_B2[2])
                        pend_B3 = (pend_B2[0], o_sb)
                        pend_B2 = None
                    if st is not None:
                        pend_B2 = (b, st, xa)

            for g in range(4):
                phase_a_group(0, g)
                ag_chunk(0, g)

            def _between_l0(b):
                if b % 5 == 4:
                    phase_a_group(1, b // 5)
                    ag_chunk(1, b // 5)

            emit_layer_edges(0, between=_between_l0)
            emit_layer_edges(1)

    nc.compile()
    return nc


_CACHE = {}


def _get_nc(e_blk, npos_key):
    key = (e_blk, npos_key)
    if key not in _CACHE:
        _CACHE[key] = _build(e_blk, [list(npos_key[0]), list(npos_key[1])])
    return _CACHE[key]


def kernel(**inputs):
    per_core, meta = _preprocess_graph(np.asarray(inputs["edge_index"]))
    wprep, npos, col_perms, invs = _prep_weights(inputs)
    e_blk = meta["e_blk"]
    perm = meta["perm"]

    nc = _get_nc(e_blk, (tuple(npos[0]), tuple(npos[1])))

    x = np.asarray(inputs["x"], np.float32)
    x_perm = x[perm].astype(bfloat16)
    in_maps = []
    for core in range(NCORES):
        xp = np.zeros((NPAD, IN), bfloat16)
        xc = x_perm[core * NSH:(core + 1) * NSH]
        xp.reshape(NBLK, 128, IN)[:, :DBLK, :] = xc.reshape(NBLK, DBLK, IN)
        m = dict(
            x_pad=xp,
            src_idx=per_core[core]["src_idx"],
            onehot=per_core[core]["onehot"],
            onehotT=per_core[core]["onehotT"],
        )
        for l in range(2):
            m[f"wl{l}"] = wprep[f"wl{l}"]
            m[f"wr{l}"] = wprep[f"wr{l}"]
            m[f"aux{l}"] = wprep[f"aux{l}"]
        in_maps.append(m)

    trace = bool(inputs.pop("_trace", False))
    res = run_bass_kernel_spmd(nc, in_maps, core_ids=list(range(NCORES)),
                               trace=trace)
    out_rows = np.concatenate([res.results[c]["out"] for c in range(NCORES)],
                              axis=0)
    tmp = np.zeros((N, HC), np.float32)
    tmp[perm] = out_rows
    out = np.zeros((N, HC), np.float32)
    bias1 = np.asarray(inputs["bias1"], np.float32)
    out[:, col_perms[1]] = tmp * invs[1][None, :] + bias1[col_perms[1]][None, :]
    if trace:
        kernel._last_result = res
    return out


# revision 20
# speedup vs baseline: 1.2590x; 1.0236x over previous
"""Trainium2 Bass kernel for a 2-layer GATv2 encoder (nn_CG_GNN_Encoder).

kernel(**inputs) takes full inputs (x [20000,512] f32, edge_index [2,320000]
int64, weights) and returns the full [20000, 512] f32 output, across 8 cores.

v3 design (per core, dst-node sharded):
  - Host: balance dst nodes into 8 cores x 20 blocks x 125 nodes; per-block
    edge lists padded to e_blk; one-hot scatter matrices in BOTH orientations
    (edge-major `oh` for value aggregation, dst-major `ohT` for broadcasting
    dst features to edges); |att| magnitudes folded into Wl/Wr columns with
    pos-att columns ordered before neg-att per head.
  - Phase A per layer: x chunks DMA-transposed, 8 matmuls per 125-node tile
    -> xl/xr [., 512], bias added during PSUM evacuation; xl stored to DRAM
    and AllGathered in 4 chunks (overlapping phase A); xr stays local.
  - Edge phase per block: ONE batched indirect gather (xl[src]) split across
    2 SWDGE queues; xr[dst] broadcast on the tensor engine (ohT matmul) with
    xl accumulated via identity matmul; LeakyReLU applied by the scalar
    engine during PSUM->SBUF evacuation.  Per-head logits = pos-column sum
    minus neg-column sum (DVE reduces), p = exp(logit); p duplicated into
    adjacent column pairs so the value multiply runs in the DVE packed 2x
    mode; one-hot matmuls accumulate values + denominators in PSUM;
    normalize, ELU between layers.  Layer-1 output unscale/bias on host.
"""

import numpy as np
from ml_dtypes import bfloat16

import concourse.bacc as bacc
import concourse.bass as bass
import concourse.mybir as mybir
import concourse.tile as tile
from concourse.bass_utils import run_bass_kernel_spmd

F32 = mybir.dt.float32
BF16 = mybir.dt.bfloat16
I16 = mybir.dt.int16
AX = mybir.AxisListType
OP = mybir.AluOpType
ACT = mybir.ActivationFunctionType

N = 20000
H = 4
C = 128
IN = 512
HC = H * C            # 512
NEG = 0.2
NCORES = 8
NSH = N // NCORES     # 2500
DBLK = 125
NBLK = NSH // DBLK    # 20
NPAD = NBLK * 128     # 2560 padded rows (tile t at rows 128t..128t+124)
AGCH = 625            # AllGather chunk rows per core (4 chunks per layer)
ATT_EPS = 1e-10


# ----------------------------------------------------------------------------
# Host-side preprocessing
# ----------------------------------------------------------------------------

def _preprocess_graph(edge_index):
    src = np.concatenate([edge_index[0], np.arange(N, dtype=np.int64)])
    dst = np.concatenate([edge_index[1], np.arange(N, dtype=np.int64)])
    deg = np.bincount(dst, minlength=N)

    nbins = NCORES * NBLK
    order = np.argsort(-deg, kind="stable")
    import heapq
    bin_load = np.zeros(nbins, np.int64)
    bin_fill = np.zeros(nbins, np.int64)
    assign = np.zeros(N, np.int64)
    heap = [(0, b) for b in range(nbins)]
    heapq.heapify(heap)
    for nid in order:
        while True:
            load, b = heapq.heappop(heap)
            if bin_fill[b] < DBLK:
                break
        assign[nid] = b
        bin_fill[b] += 1
        bin_load[b] = load + deg[nid]
        if bin_fill[b] < DBLK:
            heapq.heappush(heap, (bin_load[b], b))

    perm = np.argsort(assign * N + np.arange(N), kind="stable")
    inv_perm = np.empty(N, np.int64)
    inv_perm[perm] = np.arange(N)

    e_bin = assign[dst]
    e_dst_pos = inv_perm[dst]
    e_src_pos = inv_perm[src]
    max_per_bin = int(np.bincount(e_bin, minlength=nbins).max())
    e_blk = -(-max_per_bin // 128) * 128
    S = e_blk // 128

    order_e = np.argsort(e_bin, kind="stable")
    eb = e_bin[order_e]
    starts = np.searchsorted(eb, np.arange(nbins))
    ends = np.searchsorted(eb, np.arange(nbins), side="right")

    E16 = e_blk // 16
    src16 = np.zeros((NCORES, NBLK, 16, E16), np.int16)
    onehot = np.zeros((NCORES, 128, NBLK, S, DBLK), bfloat16)
    onehotT = np.zeros((NCORES, DBLK, NBLK, S, 128), bfloat16)

    for b in range(nbins):
        core, blk = divmod(b, NBLK)
        sel = order_e[starts[b]:ends[b]]
        n = len(sel)
        pos = np.arange(n)
        d_loc = e_dst_pos[sel] % DBLK
        # xl_full is chunk-major: AllGather chunk c (625 rows per core) is
        # contiguous as [8 cores, 625].  Map src position -> xl_full row.
        sp_ = e_src_pos[sel]
        s_core, s_r = sp_ // NSH, sp_ % NSH
        src_row = (s_r // AGCH) * (NCORES * AGCH) + s_core * AGCH + (s_r % AGCH)
        src16[core, blk, pos % 16, pos // 16] = src_row
        onehot[core, pos % 128, blk, pos // 128, d_loc] = 1.0
        onehotT[core, d_loc, blk, pos // 128, pos % 128] = 1.0

    per_core = []
    for core in range(NCORES):
        s16 = src16[core].transpose(1, 0, 2).reshape(16, -1)
        per_core.append(dict(
            src_idx=np.tile(s16, (8, 1)).copy(),
            onehot=onehot[core].reshape(128, -1).copy(),
            onehotT=onehotT[core].reshape(DBLK, -1).copy(),
        ))
    return per_core, dict(e_blk=e_blk, perm=perm)


def _prep_weights(inputs):
    """Per layer: permute columns pos-att-first per head, scale columns by
    max(|att|, eps).  logit = sum_pos lrelu(col) - sum_neg lrelu(col)."""
    out = {}
    npos = []
    col_perms = []
    invs = []
    for l in range(2):
        att = np.asarray(inputs[f"att{l}"], np.float32)
        cols = []
        np_l = []
        for h in range(H):
            pos = np.where(att[h] >= 0)[0]
            neg = np.where(att[h] < 0)[0]
            cols.append(h * C + np.concatenate([pos, neg]))
            np_l.append(len(pos))
        cols = np.concatenate(cols)
        absa = np.maximum(np.abs(att.reshape(HC)[cols]), ATT_EPS)
        col_perms.append(cols)
        npos.append(np_l)
        invs.append((1.0 / absa).astype(np.float32))

        Wl = np.asarray(inputs[f"Wl{l}"], np.float32)
        Wr = np.asarray(inputs[f"Wr{l}"], np.float32)
        bl = np.asarray(inputs[f"bl{l}"], np.float32)
        br = np.asarray(inputs[f"br{l}"], np.float32)
        if l == 1:
            Wl = Wl[col_perms[0], :]
            Wr = Wr[col_perms[0], :]
        out[f"wl{l}"] = (Wl[:, cols] * absa[None, :]).astype(bfloat16)
        out[f"wr{l}"] = (Wr[:, cols] * absa[None, :]).astype(bfloat16)
        aux = np.zeros((4, HC), np.float32)
        aux[0] = bl[cols] * absa
        aux[1] = br[cols] * absa
        aux[2] = invs[l]
        aux[3] = np.asarray(inputs[f"bias{l}"], np.float32)[cols]
        out[f"aux{l}"] = aux.astype(bfloat16)
    return out, npos, col_perms, invs


# ----------------------------------------------------------------------------
# Device kernel
# ----------------------------------------------------------------------------

def _build(e_blk, npos):
    S = e_blk // 128
    E16 = e_blk // 16
    S0 = (S + 1) // 2          # chunks handled by queue 0
    n0 = S0 * 128
    n1 = e_blk - n0
    nc = bacc.Bacc("TRN2", target_bir_lowering=False, debug=False,
                   num_devices=NCORES, num_swdge_queues=2)

    x_in = nc.dram_tensor("x_pad", [NPAD, IN], BF16, kind="ExternalInput")
    wl_d = [nc.dram_tensor(f"wl{l}", [IN, HC], BF16, kind="ExternalInput")
            for l in range(2)]
    wr_d = [nc.dram_tensor(f"wr{l}", [IN, HC], BF16, kind="ExternalInput")
            for l in range(2)]
    aux_d = [nc.dram_tensor(f"aux{l}", [4, HC], BF16, kind="ExternalInput")
             for l in range(2)]
    srcidx_d = nc.dram_tensor("src_idx", [128, NBLK * E16], I16,
                              kind="ExternalInput")
    oh_d = nc.dram_tensor("onehot", [128, NBLK * S * DBLK], BF16,
                          kind="ExternalInput")
    ohT_d = nc.dram_tensor("onehotT", [DBLK, NBLK * S * 128], BF16,
                           kind="ExternalInput")
    out_d = nc.dram_tensor("out", [NSH, HC], F32, kind="ExternalOutput")

    from concourse.masks import make_identity

    with tile.TileContext(nc) as tc:
        with tc.tile_pool(name="dram", bufs=1, space="DRAM") as dram, \
             tc.tile_pool(name="const", bufs=1) as cp, \
             tc.tile_pool(name="pha", bufs=2) as wp, \
             tc.tile_pool(name="gath", bufs=2) as gp, \
             tc.tile_pool(name="sm", bufs=2) as sp, \
             tc.tile_pool(name="psum", bufs=2, space="PSUM") as pp:

            xl_sh = [dram.tile([NSH, HC], BF16, name=f"xl_sh{l}")
                     for l in range(2)]
            xr_dr = [dram.tile([NSH, HC], BF16, name=f"xr_dr{l}")
                     for l in range(2)]
            xl_full = [dram.tile([N, HC], BF16, name=f"xl_full{l}")
                       for l in range(2)]
            h_pad = dram.tile([NPAD, HC], BF16, name="h_pad")

            ident = cp.tile([128, 128], BF16, name="ident")
            make_identity(nc, ident[:])

            si_t = cp.tile([128, NBLK * E16], I16, name="si_t")
            nc.scalar.dma_start(out=si_t[:], in_=srcidx_d[:])

            # weights + aux broadcast tiles (both layers, resident)
            wl_t, wr_t, aux_b = [], [], []
            for l in range(2):
                wlt = cp.tile([128, 4, HC], BF16, name=f"wl_t{l}")
                wrt = cp.tile([128, 4, HC], BF16, name=f"wr_t{l}")
                for k in range(4):
                    nc.scalar.dma_start(out=wlt[:, k, :],
                                        in_=wl_d[l][k * 128:(k + 1) * 128, :])
                    nc.scalar.dma_start(out=wrt[:, k, :],
                                        in_=wr_d[l][k * 128:(k + 1) * 128, :])
                wl_t.append(wlt)
                wr_t.append(wrt)
                rows = []
                for r in range(4):
                    if l == 1 and r >= 2:
                        rows.append(None)
                        continue
                    row = cp.tile([1, HC], BF16, name=f"ar{l}{r}")
                    nc.scalar.dma_start(out=row[:], in_=aux_d[l][r:r + 1, :])
                    bc = cp.tile([128, HC], BF16, name=f"ab{l}{r}")
                    nc.gpsimd.partition_broadcast(bc[:], row[:])
                    rows.append(bc)
                aux_b.append(rows)

            # ---- emit helpers ---------------------------------------------
            def phase_a_group(l, g):
                """Tiles 5g..5g+4 (640 padded rows) of phase A for layer l."""
                src_pad = x_in if l == 0 else h_pad
                xT = wp.tile([128, 4, 640], BF16, name="xT", tag="xT")
                for k in range(4):
                    nc.sync.dma_start_transpose(
                        out=xT[:, k, :],
                        in_=src_pad[g * 640:(g + 1) * 640,
                                    k * 128:(k + 1) * 128])
                for t in range(5 * g, 5 * g + 5):
                    off = (t - 5 * g) * 128
                    ps_a = pp.tile([128, 2, HC], F32, name="ps_a",
                                   tag="ps_t", bufs=3)
                    for k in range(4):
                        nc.tensor.matmul(
                            out=ps_a[0:DBLK, 0, :],
                            lhsT=xT[:, k, off:off + DBLK],
                            rhs=wl_t[l][:, k, :],
                            start=(k == 0), stop=(k == 3))
                    for k in range(4):
                        nc.tensor.matmul(
                            out=ps_a[0:DBLK, 1, :],
                            lhsT=xT[:, k, off:off + DBLK],
                            rhs=wr_t[l][:, k, :],
                            start=(k == 0), stop=(k == 3))
                    xl_o = wp.tile([DBLK, HC], BF16, name="xl_o", tag="xl_o")
                    nc.vector.tensor_add(out=xl_o[:], in0=ps_a[0:DBLK, 0, :],
                                         in1=aux_b[l][0][:DBLK, :])
                    rows = slice(t * DBLK, (t + 1) * DBLK)
                    nc.sync.dma_start(out=xl_sh[l][rows, :], in_=xl_o[:])
                    xr_o = wp.tile([DBLK, HC], BF16, name="xr_o", tag="xr_o")
                    nc.vector.tensor_add(out=xr_o[:], in0=ps_a[0:DBLK, 1, :],
                                         in1=aux_b[l][1][:DBLK, :])
                    nc.sync.dma_start(out=xr_dr[l][rows, :], in_=xr_o[:])

            def ag_chunk(l, c):
                nc.gpsimd.collective_compute(
                    "AllGather", OP.bypass,
                    replica_groups=[list(range(NCORES))],
                    ins=[xl_sh[l][c * AGCH:(c + 1) * AGCH, :]],
                    outs=[xl_full[l][c * NCORES * AGCH:
                                     (c + 1) * NCORES * AGCH, :]],
                )

            # halves: half 0 = s in [0, S0), half 1 = s in [S0, S)
            halves = [(0, S0), (S0, S)]

            def edge_load(l, b):
                """Gathers + streams for block b (both halves)."""
                io0 = b * E16
                xl_gh = []
                for hf, (sa, sb_) in enumerate(halves):
                    nh = (sb_ - sa) * 128
                    xg = gp.tile([128, S0, HC], BF16, name=f"xl_g{hf}",
                                 tag=f"xl_g{hf}", bufs=3)
                    nc.gpsimd.dma_gather(
                        out_ap=xg[:, 0:sb_ - sa, :], in_ap=xl_full[l][:],
                        idxs_ap=si_t[:, io0 + sa * 8:io0 + sa * 8 + nh // 16],
                        num_idxs=nh, num_idxs_reg=nh, elem_size=HC,
                        single_packet=False, queue_num=hf)
                    xl_gh.append(xg)

                oh_b = gp.tile([128, S, DBLK], BF16, name="oh_b", tag="oh_b")
                nc.sync.dma_start(
                    out=oh_b[:],
                    in_=oh_d[:, b * S * DBLK:(b + 1) * S * DBLK])
                ohT_b = gp.tile([DBLK, S, 128], BF16, name="ohT_b",
                                tag="ohT_b")
                nc.sync.dma_start(
                    out=ohT_b[:],
                    in_=ohT_d[:, b * S * 128:(b + 1) * S * 128])
                xr_b = gp.tile([DBLK, HC], BF16, name="xr_b", tag="xr_b")
                nc.sync.dma_start(
                    out=xr_b[:], in_=xr_dr[l][b * DBLK:(b + 1) * DBLK, :])
                return xl_gh, oh_b, ohT_b, xr_b

            def edge_mm(l, b, ld):
                """t = xr[dst] + xl[src] matmuls into PSUM."""
                xl_gh, oh_b, ohT_b, xr_b = ld
                ps_lists = [[], []]
                for hf, (sa, sb_) in enumerate(halves):
                    for s2 in range(sa, sb_, 2):
                        w = min(2, sb_ - s2)
                        ps_t = pp.tile([128, 2, HC], F32, name="ps_t",
                                       tag="ps_t", bufs=3)
                        for s in range(s2, s2 + w):
                            nc.tensor.matmul(out=ps_t[:, s - s2, :],
                                             lhsT=ohT_b[:, s, :], rhs=xr_b[:],
                                             start=True, stop=False)
                            nc.tensor.matmul(
                                out=ps_t[:, s - s2, :], lhsT=ident[:],
                                rhs=xl_gh[hf][:, s - sa, :],
                                start=False, stop=True)
                        ps_lists[hf].append((s2, w, ps_t))
                return xl_gh, oh_b, ps_lists

            def edge_evac(l, b, st):
                """ACT LeakyReLU evacuation PSUM -> bf16 SBUF, per half."""
                xl_gh, oh_b, ps_lists = st
                t_lrh = []
                for hf, (sa, sb_) in enumerate(halves):
                    t_lr = gp.tile([128, S0, HC], BF16, name=f"t_lr{hf}",
                                   tag=f"t_lr{hf}")
                    for s2, w, ps_t in ps_lists[hf]:
                        nc.scalar.activation(
                            out=t_lr[:, s2 - sa:s2 - sa + w, :],
                            in_=ps_t[:, 0:w, :], func=ACT.Prelu, alpha=NEG)
                    t_lrh.append(t_lr)
                return t_lrh

            def edge_B1(l, b, st, t_lrh):
                """Per half: logit reduces + exp + value multiply."""
                xl_gh, oh_b, ps_lists = st
                xa = gp.tile([128, S, 2, 258], BF16, name="xa", tag="xa")
                for hf, (sa, sb_) in enumerate(halves):
                    ns = sb_ - sa
                    t_lr = t_lrh[hf]
                    lg_pn = sp.tile([128, 2, S0, H], F32, name=f"lg_pn{hf}",
                                    tag=f"lg_pn{hf}")
                    for h in range(H):
                        np_h = npos[l][h]
                        lo, mid, hi = h * C, h * C + np_h, (h + 1) * C
                        if np_h > 0:
                            nc.vector.tensor_reduce(
                                out=lg_pn[:, 0, 0:ns, h],
                                in_=t_lr[:, 0:ns, lo:mid],
                                axis=AX.X, op=OP.add)
                        else:
                            nc.vector.memset(lg_pn[:, 0, 0:ns, h], 0.0)
                        if np_h < C:
                            nc.vector.tensor_reduce(
                                out=lg_pn[:, 1, 0:ns, h],
                                in_=t_lr[:, 0:ns, mid:hi],
                                axis=AX.X, op=OP.add)
                        else:
                            nc.vector.memset(lg_pn[:, 1, 0:ns, h], 0.0)
                    lg = sp.tile([128, S0, H], F32, name=f"lg{hf}",
                                 tag=f"lg{hf}")
                    nc.vector.tensor_tensor(out=lg[:, 0:ns, :],
                                            in0=lg_pn[:, 0, 0:ns, :],
                                            in1=lg_pn[:, 1, 0:ns, :],
                                            op=OP.subtract)
                    # p = exp(lg): denominator cols + duplicated pairs
                    nc.scalar.activation(
                        out=xa[:, sa:sb_, :, 256:258],
                        in_=lg[:, 0:ns, :].rearrange(
                            "p s (a b) -> p s a b", a=2),
                        func=ACT.Exp)
                    p_dup = sp.tile([128, S0, H, 2], BF16, name=f"p_dup{hf}",
                                    tag=f"p_dup{hf}")
                    nc.scalar.activation(out=p_dup[:, 0:ns, :, 0],
                                         in_=lg[:, 0:ns, :], func=ACT.Exp)
                    nc.scalar.activation(out=p_dup[:, 0:ns, :, 1],
                                         in_=lg[:, 0:ns, :], func=ACT.Exp)
                    # xa = xl * p (packed 2x)
                    for h in range(H):
                        hp, hh = divmod(h, 2)
                        nc.vector.tensor_tensor(
                            out=xa[:, sa:sb_, hp,
                                   hh * 128:(hh + 1) * 128].rearrange(
                                "p s (pr two) -> p s pr two", two=2),
                            in0=xl_gh[hf][:, 0:ns,
                                          h * 128:(h + 1) * 128].rearrange(
                                "p s (pr two) -> p s pr two", two=2),
                            in1=p_dup[:, 0:ns, h, None, :]
                                .to_broadcast([128, ns, 64, 2]),
                            op=OP.mult)
                return xa

            def edge_B2(l, b, st, xa):
                """Value one-hot matmuls + normalize; ELU/store deferred."""
                xl_gh, oh_b, ps_lists = st
                ps_o = [pp.tile([DBLK, HC], F32, name=f"ps_o{hp}",
                                tag="ps_sh", bufs=2) for hp in range(2)]
                for s in range(S):
                    for hp in range(2):
                        nc.tensor.matmul(out=ps_o[hp][:, 0:258],
                                         lhsT=oh_b[:, s, :],
                                         rhs=xa[:, s, hp, :],
                                         start=(s == 0), stop=(s == S - 1))

                rinv = sp.tile([DBLK, 4], F32, name="rinv", tag="rinv")
                for hp in range(2):
                    nc.vector.reciprocal(out=rinv[:, 2 * hp:2 * hp + 2],
                                         in_=ps_o[hp][:, 256:258])
                o_sb = sp.tile([DBLK, HC], BF16 if l == 0 else F32,
                               name="o_sb", tag=f"o_sb{l}")
                for hp in range(2):
                    nc.vector.tensor_tensor(
                        out=o_sb[:, hp * 256:(hp + 1) * 256].rearrange(
                            "p (h c) -> p h c", h=2),
                        in0=ps_o[hp][:, 0:256].rearrange(
                            "p (h c) -> p h c", h=2),
                        in1=rinv[:, 2 * hp:2 * hp + 2][:, :, None]
                            .to_broadcast([DBLK, 2, 128]),
                        op=OP.mult)
                return o_sb

            def edge_B3(l, b, o_sb):
                """ELU (layer 0) and store for block b."""
                if l == 0:
                    nc.vector.tensor_mul(out=o_sb[:], in0=o_sb[:],
                                         in1=aux_b[0][2][:DBLK, :])
                    nc.vector.tensor_add(out=o_sb[:], in0=o_sb[:],
                                         in1=aux_b[0][3][:DBLK, :])
                    r_t = sp.tile([DBLK, HC], BF16, name="r_t", tag="r_t")
                    nc.scalar.activation(out=r_t[:], in_=o_sb[:],
                                         func=ACT.Relu)
                    e_t = sp.tile([DBLK, HC], BF16, name="e_t", tag="e_t")
                    nc.scalar.activation(out=e_t[:], in_=o_sb[:],
                                         func=ACT.Exp)
                    nc.vector.tensor_scalar(
                        out=e_t[:], in0=e_t[:], scalar1=-1.0, scalar2=0.0,
                        op0=OP.add, op1=OP.min)
                    h_t = sp.tile([DBLK, HC], BF16, name="h_t", tag="h_t")
                    nc.vector.tensor_add(out=h_t[:], in0=r_t[:], in1=e_t[:])
                    nc.sync.dma_start(
                        out=h_pad[b * 128:b * 128 + DBLK, :], in_=h_t[:])
                else:
                    nc.sync.dma_start(
                        out=out_d[b * DBLK:(b + 1) * DBLK, :], in_=o_sb[:])

            # ---- schedule (software pipeline per layer) -------------------
            def emit_layer_edges(l, between=None):
                pend_B2 = None   # (b, st, xa)
                pend_B3 = None   # (b, o_sb)
                ld = edge_load(l, 0)
                for b in range(NBLK + 2):
                    st = t_lrh = None
                    if b < NBLK:
                        st = edge_mm(l, b, ld)
                        t_lrh = edge_evac(l, b, st)
                    if b + 1 < NBLK:
                        ld = edge_load(l, b + 1)
                    if pend_B3 is not None:
                        edge_B3(l, *pend_B3)
                        if between is not None:
                            between(pend_B3[0])
                        pend_B3 = None
                    if st is not None:
                        xa = edge_B1(l, b, st, t_lrh)
                    if pend_B2 is not None:
                        o_sb = edge_B2(l, pend_B2[0], pend_B2[1],
                                       pend_B2[2])
                        pend_B3 = (pend_B2[0], o_sb)
                        pend_B2 = None
                    if st is not None:
                        pend_B2 = (b, st, xa)

            for g in range(4):
                phase_a_group(0, g)
                ag_chunk(0, g)

            def _between_l0(b):
                if b % 5 == 4:
                    phase_a_group(1, b // 5)
                    ag_chunk(1, b // 5)

            emit_layer_edges(0, between=_between_l0)
            emit_layer_edges(1)

    nc.compile()
    return nc


_CACHE = {}


def _get_nc(e_blk, npos_key):
    key = (e_blk, npos_key)
    if key not in _CACHE:
        _CACHE[key] = _build(e_blk, [list(npos_key[0]), list(npos_key[1])])
    return _CACHE[key]


def kernel(**inputs):
    per_core, meta = _preprocess_graph(np.asarray(inputs["edge_index"]))
    wprep, npos, col_perms, invs = _prep_weights(inputs)
    e_blk = meta["e_blk"]
    perm = meta["perm"]

    nc = _get_nc(e_blk, (tuple(npos[0]), tuple(npos[1])))

    x = np.asarray(inputs["x"], np.float32)
    x_perm = x[perm].astype(bfloat16)
    in_maps = []
    for core in range(NCORES):
        xp = np.zeros((NPAD, IN), bfloat16)
        xc = x_perm[core * NSH:(core + 1) * NSH]
        xp.reshape(NBLK, 128, IN)[:, :DBLK, :] = xc.reshape(NBLK, DBLK, IN)
        m = dict(
            x_pad=xp,
            src_idx=per_core[core]["src_idx"],
            onehot=per_core[core]["onehot"],
            onehotT=per_core[core]["onehotT"],
        )
        for l in range(2):
            m[f"wl{l}"] = wprep[f"wl{l}"]
            m[f"wr{l}"] = wprep[f"wr{l}"]
            m[f"aux{l}"] = wprep[f"aux{l}"]
        in_maps.append(m)

    trace = bool(inputs.pop("_trace", False))
    res = run_bass_kernel_spmd(nc, in_maps, core_ids=list(range(NCORES)),
                               trace=trace)
    out_rows = np.concatenate([res.results[c]["out"] for c in range(NCORES)],
                              axis=0)
    tmp = np.zeros((N, HC), np.float32)
    tmp[perm] = out_rows
    out = np.zeros((N, HC), np.float32)
    bias1 = np.asarray(inputs["bias1"], np.float32)
    out[:, col_perms[1]] = tmp * invs[1][None, :] + bias1[col_perms[1]][None, :]
    if trace:
        kernel._last_result = res
    return out


# revision 21
# speedup vs baseline: 1.3733x; 1.0908x over previous
"""Trainium2 Bass kernel for a 2-layer GATv2 encoder (nn_CG_GNN_Encoder).

kernel(**inputs) takes full inputs (x [20000,512] f32, edge_index [2,320000]
int64, weights) and returns the full [20000, 512] f32 output, across 8 cores.

v3 design (per core, dst-node sharded):
  - Host: balance dst nodes into 8 cores x 20 blocks x 125 nodes; per-block
    edge lists padded to e_blk; one-hot scatter matrices in BOTH orientations
    (edge-major `oh` for value aggregation, dst-major `ohT` for broadcasting
    dst features to edges); |att| magnitudes folded into Wl/Wr columns with
    pos-att columns ordered before neg-att per head.
  - Phase A per layer: x chunks DMA-transposed, 8 matmuls per 125-node tile
    -> xl/xr [., 512], bias added during PSUM evacuation; xl stored to DRAM
    and AllGathered in 4 chunks (overlapping phase A); xr stays local.
  - Edge phase per block: ONE batched indirect gather (xl[src]) split across
    2 SWDGE queues; xr[dst] broadcast on the tensor engine (ohT matmul) with
    xl accumulated via identity matmul; LeakyReLU applied by the scalar
    engine during PSUM->SBUF evacuation.  Per-head logits = pos-column sum
    minus neg-column sum (DVE reduces), p = exp(logit); p duplicated into
    adjacent column pairs so the value multiply runs in the DVE packed 2x
    mode; one-hot matmuls accumulate values + denominators in PSUM;
    normalize, ELU between layers.  Layer-1 output unscale/bias on host.
"""

import numpy as np
from ml_dtypes import bfloat16

import concourse.bacc as bacc
import concourse.bass as bass
import concourse.mybir as mybir
import concourse.tile as tile
from concourse.bass_utils import run_bass_kernel_spmd

F32 = mybir.dt.float32
BF16 = mybir.dt.bfloat16
I16 = mybir.dt.int16
AX = mybir.AxisListType
OP = mybir.AluOpType
ACT = mybir.ActivationFunctionType

N = 20000
H = 4
C = 128
IN = 512
HC = H * C            # 512
NEG = 0.2
NCORES = 8
NSH = N // NCORES     # 2500
DBLK = 125
NBLK = NSH // DBLK    # 20
NPAD = NBLK * 128     # 2560 padded rows (tile t at rows 128t..128t+124)
AGCH = 625            # AllGather chunk rows per core (4 chunks per layer)
ATT_EPS = 1e-10


# ----------------------------------------------------------------------------
# Host-side preprocessing
# ----------------------------------------------------------------------------

def _preprocess_graph(edge_index):
    src = np.concatenate([edge_index[0], np.arange(N, dtype=np.int64)])
    dst = np.concatenate([edge_index[1], np.arange(N, dtype=np.int64)])
    deg = np.bincount(dst, minlength=N)

    nbins = NCORES * NBLK
    order = np.argsort(-deg, kind="stable")
    import heapq
    bin_load = np.zeros(nbins, np.int64)
    bin_fill = np.zeros(nbins, np.int64)
    assign = np.zeros(N, np.int64)
    heap = [(0, b) for b in range(nbins)]
    heapq.heapify(heap)
    for nid in order:
        while True:
            load, b = heapq.heappop(heap)
            if bin_fill[b] < DBLK:
                break
        assign[nid] = b
        bin_fill[b] += 1
        bin_load[b] = load + deg[nid]
        if bin_fill[b] < DBLK:
            heapq.heappush(heap, (bin_load[b], b))

    perm = np.argsort(assign * N + np.arange(N), kind="stable")
    inv_perm = np.empty(N, np.int64)
    inv_perm[perm] = np.arange(N)

    e_bin = assign[dst]
    e_dst_pos = inv_perm[dst]
    e_src_pos = inv_perm[src]
    max_per_bin = int(np.bincount(e_bin, minlength=nbins).max())
    e_blk = -(-max_per_bin // 128) * 128
    S = e_blk // 128

    order_e = np.argsort(e_bin, kind="stable")
    eb = e_bin[order_e]
    starts = np.searchsorted(eb, np.arange(nbins))
    ends = np.searchsorted(eb, np.arange(nbins), side="right")

    E16 = e_blk // 16
    src16 = np.zeros((NCORES, NBLK, 16, E16), np.int16)
    onehot = np.zeros((NCORES, 128, NBLK, S, DBLK), bfloat16)
    onehotT = np.zeros((NCORES, DBLK, NBLK, S, 128), bfloat16)

    for b in range(nbins):
        core, blk = divmod(b, NBLK)
        sel = order_e[starts[b]:ends[b]]
        n = len(sel)
        pos = np.arange(n)
        d_loc = e_dst_pos[sel] % DBLK
        # xl_full is chunk-major: AllGather chunk c (625 rows per core) is
        # contiguous as [8 cores, 625].  Map src position -> xl_full row.
        sp_ = e_src_pos[sel]
        s_core, s_r = sp_ // NSH, sp_ % NSH
        src_row = (s_r // AGCH) * (NCORES * AGCH) + s_core * AGCH + (s_r % AGCH)
        src16[core, blk, pos % 16, pos // 16] = src_row
        onehot[core, pos % 128, blk, pos // 128, d_loc] = 1.0
        onehotT[core, d_loc, blk, pos // 128, pos % 128] = 1.0

    per_core = []
    for core in range(NCORES):
        s16 = src16[core].transpose(1, 0, 2).reshape(16, -1)
        per_core.append(dict(
            src_idx=np.tile(s16, (8, 1)).copy(),
            onehot=onehot[core].reshape(128, -1).copy(),
            onehotT=onehotT[core].reshape(DBLK, -1).copy(),
        ))
    return per_core, dict(e_blk=e_blk, perm=perm)


def _prep_weights(inputs):
    """Per layer: permute columns pos-att-first per head, scale columns by
    max(|att|, eps).  logit = sum_pos lrelu(col) - sum_neg lrelu(col)."""
    out = {}
    npos = []
    col_perms = []
    invs = []
    for l in range(2):
        att = np.asarray(inputs[f"att{l}"], np.float32)
        cols = []
        np_l = []
        for h in range(H):
            pos = np.where(att[h] >= 0)[0]
            neg = np.where(att[h] < 0)[0]
            cols.append(h * C + np.concatenate([pos, neg]))
            np_l.append(len(pos))
        cols = np.concatenate(cols)
        absa = np.maximum(np.abs(att.reshape(HC)[cols]), ATT_EPS)
        col_perms.append(cols)
        npos.append(np_l)
        invs.append((1.0 / absa).astype(np.float32))

        Wl = np.asarray(inputs[f"Wl{l}"], np.float32)
        Wr = np.asarray(inputs[f"Wr{l}"], np.float32)
        bl = np.asarray(inputs[f"bl{l}"], np.float32)
        br = np.asarray(inputs[f"br{l}"], np.float32)
        if l == 1:
            Wl = Wl[col_perms[0], :]
            Wr = Wr[col_perms[0], :]
        out[f"wl{l}"] = (Wl[:, cols] * absa[None, :]).astype(bfloat16)
        out[f"wr{l}"] = (Wr[:, cols] * absa[None, :]).astype(bfloat16)
        aux = np.zeros((4, HC), np.float32)
        aux[0] = bl[cols] * absa
        aux[1] = br[cols] * absa
        aux[2] = invs[l]
        aux[3] = np.asarray(inputs[f"bias{l}"], np.float32)[cols]
        out[f"aux{l}"] = aux.astype(bfloat16)
    return out, npos, col_perms, invs


# ----------------------------------------------------------------------------
# Device kernel
# ----------------------------------------------------------------------------

def _build(e_blk, npos):
    S = e_blk // 128
    E16 = e_blk // 16
    S0 = (S + 1) // 2          # chunks handled by queue 0
    n0 = S0 * 128
    n1 = e_blk - n0
    nc = bacc.Bacc("TRN2", target_bir_lowering=False, debug=False,
                   num_devices=NCORES, num_swdge_queues=4)

    x_in = nc.dram_tensor("x_pad", [NPAD, IN], BF16, kind="ExternalInput")
    wl_d = [nc.dram_tensor(f"wl{l}", [IN, HC], BF16, kind="ExternalInput")
            for l in range(2)]
    wr_d = [nc.dram_tensor(f"wr{l}", [IN, HC], BF16, kind="ExternalInput")
            for l in range(2)]
    aux_d = [nc.dram_tensor(f"aux{l}", [4, HC], BF16, kind="ExternalInput")
             for l in range(2)]
    srcidx_d = nc.dram_tensor("src_idx", [128, NBLK * E16], I16,
                              kind="ExternalInput")
    oh_d = nc.dram_tensor("onehot", [128, NBLK * S * DBLK], BF16,
                          kind="ExternalInput")
    ohT_d = nc.dram_tensor("onehotT", [DBLK, NBLK * S * 128], BF16,
                           kind="ExternalInput")
    out_d = nc.dram_tensor("out", [NSH, HC], F32, kind="ExternalOutput")

    from concourse.masks import make_identity

    with tile.TileContext(nc) as tc:
        with tc.tile_pool(name="dram", bufs=1, space="DRAM") as dram, \
             tc.tile_pool(name="const", bufs=1) as cp, \
             tc.tile_pool(name="pha", bufs=2) as wp, \
             tc.tile_pool(name="gath", bufs=2) as gp, \
             tc.tile_pool(name="sm", bufs=2) as sp, \
             tc.tile_pool(name="psum", bufs=2, space="PSUM") as pp:

            xl_sh = [dram.tile([NSH, HC], BF16, name=f"xl_sh{l}")
                     for l in range(2)]
            xr_dr = [dram.tile([NSH, HC], BF16, name=f"xr_dr{l}")
                     for l in range(2)]
            xl_full = [dram.tile([N, HC], BF16, name=f"xl_full{l}")
                       for l in range(2)]
            h_pad = dram.tile([NPAD, HC], BF16, name="h_pad")

            ident = cp.tile([128, 128], BF16, name="ident")
            make_identity(nc, ident[:])

            si_t = cp.tile([128, NBLK * E16], I16, name="si_t")
            nc.scalar.dma_start(out=si_t[:], in_=srcidx_d[:])

            # weights + aux broadcast tiles (both layers, resident)
            wl_t, wr_t, aux_b = [], [], []
            for l in range(2):
                wlt = cp.tile([128, 4, HC], BF16, name=f"wl_t{l}")
                wrt = cp.tile([128, 4, HC], BF16, name=f"wr_t{l}")
                for k in range(4):
                    nc.scalar.dma_start(out=wlt[:, k, :],
                                        in_=wl_d[l][k * 128:(k + 1) * 128, :])
                    nc.scalar.dma_start(out=wrt[:, k, :],
                                        in_=wr_d[l][k * 128:(k + 1) * 128, :])
                wl_t.append(wlt)
                wr_t.append(wrt)
                rows = []
                for r in range(4):
                    if l == 1 and r >= 2:
                        rows.append(None)
                        continue
                    row = cp.tile([1, HC], BF16, name=f"ar{l}{r}")
                    nc.scalar.dma_start(out=row[:], in_=aux_d[l][r:r + 1, :])
                    bc = cp.tile([128, HC], BF16, name=f"ab{l}{r}")
                    nc.gpsimd.partition_broadcast(bc[:], row[:])
                    rows.append(bc)
                aux_b.append(rows)

            # ---- emit helpers ---------------------------------------------
            def phase_a_group(l, g):
                """Tiles 5g..5g+4 (640 padded rows) of phase A for layer l."""
                src_pad = x_in if l == 0 else h_pad
                xT = wp.tile([128, 4, 640], BF16, name="xT", tag="xT")
                for k in range(4):
                    nc.sync.dma_start_transpose(
                        out=xT[:, k, :],
                        in_=src_pad[g * 640:(g + 1) * 640,
                                    k * 128:(k + 1) * 128])
                for t in range(5 * g, 5 * g + 5):
                    off = (t - 5 * g) * 128
                    ps_a = pp.tile([128, 2, HC], F32, name="ps_a",
                                   tag="ps_t", bufs=3)
                    for k in range(4):
                        nc.tensor.matmul(
                            out=ps_a[0:DBLK, 0, :],
                            lhsT=xT[:, k, off:off + DBLK],
                            rhs=wl_t[l][:, k, :],
                            start=(k == 0), stop=(k == 3))
                    for k in range(4):
                        nc.tensor.matmul(
                            out=ps_a[0:DBLK, 1, :],
                            lhsT=xT[:, k, off:off + DBLK],
                            rhs=wr_t[l][:, k, :],
                            start=(k == 0), stop=(k == 3))
                    xl_o = wp.tile([DBLK, HC], BF16, name="xl_o", tag="xl_o")
                    nc.vector.tensor_add(out=xl_o[:], in0=ps_a[0:DBLK, 0, :],
                                         in1=aux_b[l][0][:DBLK, :])
                    rows = slice(t * DBLK, (t + 1) * DBLK)
                    nc.sync.dma_start(out=xl_sh[l][rows, :], in_=xl_o[:])
                    xr_o = wp.tile([DBLK, HC], BF16, name="xr_o", tag="xr_o")
                    nc.vector.tensor_add(out=xr_o[:], in0=ps_a[0:DBLK, 1, :],
                                         in1=aux_b[l][1][:DBLK, :])
                    nc.sync.dma_start(out=xr_dr[l][rows, :], in_=xr_o[:])

            def ag_chunk(l, c):
                nc.gpsimd.collective_compute(
                    "AllGather", OP.bypass,
                    replica_groups=[list(range(NCORES))],
                    ins=[xl_sh[l][c * AGCH:(c + 1) * AGCH, :]],
                    outs=[xl_full[l][c * NCORES * AGCH:
                                     (c + 1) * NCORES * AGCH, :]],
                )

            # halves: half 0 = s in [0, S0), half 1 = s in [S0, S)
            halves = [(0, S0), (S0, S)]

            def edge_load(l, b):
                """Gathers + streams for block b (both halves)."""
                io0 = b * E16
                xl_gh = []
                for hf, (sa, sb_) in enumerate(halves):
                    ns = sb_ - sa
                    xg = gp.tile([128, S0, HC], BF16, name=f"xl_g{hf}",
                                 tag=f"xl_g{hf}", bufs=3)
                    mid = (ns + 1) // 2
                    for qi, (qa, qb) in enumerate(((0, mid), (mid, ns))):
                        nq_ = (qb - qa) * 128
                        nc.gpsimd.dma_gather(
                            out_ap=xg[:, qa:qb, :], in_ap=xl_full[l][:],
                            idxs_ap=si_t[:, io0 + (sa + qa) * 8:
                                         io0 + (sa + qa) * 8 + nq_ // 16],
                            num_idxs=nq_, num_idxs_reg=nq_, elem_size=HC,
                            single_packet=False, queue_num=hf * 2 + qi)
                    xl_gh.append(xg)

                oh_b = gp.tile([128, S, DBLK], BF16, name="oh_b", tag="oh_b")
                nc.sync.dma_start(
                    out=oh_b[:],
                    in_=oh_d[:, b * S * DBLK:(b + 1) * S * DBLK])
                ohT_b = gp.tile([DBLK, S, 128], BF16, name="ohT_b",
                                tag="ohT_b")
                nc.sync.dma_start(
                    out=ohT_b[:],
                    in_=ohT_d[:, b * S * 128:(b + 1) * S * 128])
                xr_b = gp.tile([DBLK, HC], BF16, name="xr_b", tag="xr_b")
                nc.sync.dma_start(
                    out=xr_b[:], in_=xr_dr[l][b * DBLK:(b + 1) * DBLK, :])
                return xl_gh, oh_b, ohT_b, xr_b

            def edge_mm(l, b, ld):
                """t = xr[dst] + xl[src] matmuls into PSUM."""
                xl_gh, oh_b, ohT_b, xr_b = ld
                ps_lists = [[], []]
                for hf, (sa, sb_) in enumerate(halves):
                    for s2 in range(sa, sb_, 2):
                        w = min(2, sb_ - s2)
                        ps_t = pp.tile([128, 2, HC], F32, name="ps_t",
                                       tag="ps_t", bufs=3)
                        for s in range(s2, s2 + w):
                            nc.tensor.matmul(out=ps_t[:, s - s2, :],
                                             lhsT=ohT_b[:, s, :], rhs=xr_b[:],
                                             start=True, stop=False)
                            nc.tensor.matmul(
                                out=ps_t[:, s - s2, :], lhsT=ident[:],
                                rhs=xl_gh[hf][:, s - sa, :],
                                start=False, stop=True)
                        ps_lists[hf].append((s2, w, ps_t))
                return xl_gh, oh_b, ps_lists

            def edge_evac(l, b, st):
                """ACT LeakyReLU evacuation PSUM -> bf16 SBUF, per half."""
                xl_gh, oh_b, ps_lists = st
                t_lrh = []
                for hf, (sa, sb_) in enumerate(halves):
                    t_lr = gp.tile([128, S0, HC], BF16, name=f"t_lr{hf}",
                                   tag=f"t_lr{hf}")
                    for s2, w, ps_t in ps_lists[hf]:
                        nc.scalar.activation(
                            out=t_lr[:, s2 - sa:s2 - sa + w, :],
                            in_=ps_t[:, 0:w, :], func=ACT.Prelu, alpha=NEG)
                    t_lrh.append(t_lr)
                return t_lrh

            def edge_B1(l, b, st, t_lrh):
                """Per half: logit reduces + exp + value multiply."""
                xl_gh, oh_b, ps_lists = st
                xa = gp.tile([128, S, 2, 258], BF16, name="xa", tag="xa")
                for hf, (sa, sb_) in enumerate(halves):
                    ns = sb_ - sa
                    t_lr = t_lrh[hf]
                    lg_pn = sp.tile([128, 2, S0, H], F32, name=f"lg_pn{hf}",
                                    tag=f"lg_pn{hf}")
                    for h in range(H):
                        np_h = npos[l][h]
                        lo, mid, hi = h * C, h * C + np_h, (h + 1) * C
                        if np_h > 0:
                            nc.vector.tensor_reduce(
                                out=lg_pn[:, 0, 0:ns, h],
                                in_=t_lr[:, 0:ns, lo:mid],
                                axis=AX.X, op=OP.add)
                        else:
                            nc.vector.memset(lg_pn[:, 0, 0:ns, h], 0.0)
                        if np_h < C:
                            nc.vector.tensor_reduce(
                                out=lg_pn[:, 1, 0:ns, h],
                                in_=t_lr[:, 0:ns, mid:hi],
                                axis=AX.X, op=OP.add)
                        else:
                            nc.vector.memset(lg_pn[:, 1, 0:ns, h], 0.0)
                    lg = sp.tile([128, S0, H], F32, name=f"lg{hf}",
                                 tag=f"lg{hf}")
                    nc.vector.tensor_tensor(out=lg[:, 0:ns, :],
                                            in0=lg_pn[:, 0, 0:ns, :],
                                            in1=lg_pn[:, 1, 0:ns, :],
                                            op=OP.subtract)
                    # p = exp(lg): denominator cols + duplicated pairs
                    nc.scalar.activation(
                        out=xa[:, sa:sb_, :, 256:258],
                        in_=lg[:, 0:ns, :].rearrange(
                            "p s (a b) -> p s a b", a=2),
                        func=ACT.Exp)
                    p_dup = sp.tile([128, S0, H, 2], BF16, name=f"p_dup{hf}",
                                    tag=f"p_dup{hf}")
                    nc.scalar.activation(out=p_dup[:, 0:ns, :, 0],
                                         in_=lg[:, 0:ns, :], func=ACT.Exp)
                    nc.scalar.activation(out=p_dup[:, 0:ns, :, 1],
                                         in_=lg[:, 0:ns, :], func=ACT.Exp)
                    # xa = xl * p (packed 2x)
                    for h in range(H):
                        hp, hh = divmod(h, 2)
                        nc.vector.tensor_tensor(
                            out=xa[:, sa:sb_, hp,
                                   hh * 128:(hh + 1) * 128].rearrange(
                                "p s (pr two) -> p s pr two", two=2),
                            in0=xl_gh[hf][:, 0:ns,
                                          h * 128:(h + 1) * 128].rearrange(
                                "p s (pr two) -> p s pr two", two=2),
                            in1=p_dup[:, 0:ns, h, None, :]
                                .to_broadcast([128, ns, 64, 2]),
                            op=OP.mult)
                return xa

            def edge_B2(l, b, st, xa):
                """Value one-hot matmuls + normalize; ELU/store deferred."""
                xl_gh, oh_b, ps_lists = st
                ps_o = [pp.tile([DBLK, HC], F32, name=f"ps_o{hp}",
                                tag="ps_sh", bufs=2) for hp in range(2)]
                for s in range(S):
                    for hp in range(2):
                        nc.tensor.matmul(out=ps_o[hp][:, 0:258],
                                         lhsT=oh_b[:, s, :],
                                         rhs=xa[:, s, hp, :],
                                         start=(s == 0), stop=(s == S - 1))

                rinv = sp.tile([DBLK, 4], F32, name="rinv", tag="rinv")
                for hp in range(2):
                    nc.vector.reciprocal(out=rinv[:, 2 * hp:2 * hp + 2],
                                         in_=ps_o[hp][:, 256:258])
                o_sb = sp.tile([DBLK, HC], BF16 if l == 0 else F32,
                               name="o_sb", tag=f"o_sb{l}")
                for hp in range(2):
                    nc.vector.tensor_tensor(
                        out=o_sb[:, hp * 256:(hp + 1) * 256].rearrange(
                            "p (h c) -> p h c", h=2),
                        in0=ps_o[hp][:, 0:256].rearrange(
                            "p (h c) -> p h c", h=2),
                        in1=rinv[:, 2 * hp:2 * hp + 2][:, :, None]
                            .to_broadcast([DBLK, 2, 128]),
                        op=OP.mult)
                return o_sb

            def edge_B3(l, b, o_sb):
                """ELU (layer 0) and store for block b."""
                if l == 0:
                    nc.vector.tensor_mul(out=o_sb[:], in0=o_sb[:],
                                         in1=aux_b[0][2][:DBLK, :])
                    nc.vector.tensor_add(out=o_sb[:], in0=o_sb[:],
                                         in1=aux_b[0][3][:DBLK, :])
                    r_t = sp.tile([DBLK, HC], BF16, name="r_t", tag="r_t")
                    nc.scalar.activation(out=r_t[:], in_=o_sb[:],
                                         func=ACT.Relu)
                    e_t = sp.tile([DBLK, HC], BF16, name="e_t", tag="e_t")
                    nc.scalar.activation(out=e_t[:], in_=o_sb[:],
                                         func=ACT.Exp)
                    nc.vector.tensor_scalar(
                        out=e_t[:], in0=e_t[:], scalar1=-1.0, scalar2=0.0,
                        op0=OP.add, op1=OP.min)
                    h_t = sp.tile([DBLK, HC], BF16, name="h_t", tag="h_t")
                    nc.vector.tensor_add(out=h_t[:], in0=r_t[:], in1=e_t[:])
                    nc.sync.dma_start(
                        out=h_pad[b * 128:b * 128 + DBLK, :], in_=h_t[:])
                else:
                    nc.sync.dma_start(
                        out=out_d[b * DBLK:(b + 1) * DBLK, :], in_=o_sb[:])

            # ---- schedule (software pipeline per layer) -------------------
            def emit_layer_edges(l, between=None):
                pend_B2 = None   # (b, st, xa)
                pend_B3 = None   # (b, o_sb)
                ld = edge_load(l, 0)
                for b in range(NBLK + 2):
                    st = t_lrh = None
                    if b < NBLK:
                        st = edge_mm(l, b, ld)
                        t_lrh = edge_evac(l, b, st)
                    if b + 1 < NBLK:
                        ld = edge_load(l, b + 1)
                    if st is not None:
                        xa = edge_B1(l, b, st, t_lrh)
                    if pend_B3 is not None:
                        edge_B3(l, *pend_B3)
                        if between is not None:
                            between(pend_B3[0])
                        pend_B3 = None
                    if pend_B2 is not None:
                        o_sb = edge_B2(l, pend_B2[0], pend_B2[1],
                                       pend_B2[2])
                        pend_B3 = (pend_B2[0], o_sb)
                        pend_B2 = None
                    if st is not None:
                        pend_B2 = (b, st, xa)

            for g in range(4):
                phase_a_group(0, g)
                ag_chunk(0, g)

            def _between_l0(b):
                if b % 5 == 4:
                    phase_a_group(1, b // 5)
                    ag_chunk(1, b // 5)

            emit_layer_edges(0, between=_between_l0)
            emit_layer_edges(1)

    nc.compile()
    return nc


_CACHE = {}


def _get_nc(e_blk, npos_key):
    key = (e_blk, npos_key)
    if key not in _CACHE:
        _CACHE[key] = _build(e_blk, [list(npos_key[0]), list(npos_key[1])])
    return _CACHE[key]


def kernel(**inputs):
    per_core, meta = _preprocess_graph(np.asarray(inputs["edge_index"]))
    wprep, npos, col_perms, invs = _prep_weights(inputs)
    e_blk = meta["e_blk"]
    perm = meta["perm"]

    nc = _get_nc(e_blk, (tuple(npos[0]), tuple(npos[1])))

    x = np.asarray(inputs["x"], np.float32)
    x_perm = x[perm].astype(bfloat16)
    in_maps = []
    for core in range(NCORES):
        xp = np.zeros((NPAD, IN), bfloat16)
        xc = x_perm[core * NSH:(core + 1) * NSH]
        xp.reshape(NBLK, 128, IN)[:, :DBLK, :] = xc.reshape(NBLK, DBLK, IN)
        m = dict(
            x_pad=xp,
            src_idx=per_core[core]["src_idx"],
            onehot=per_core[core]["onehot"],
            onehotT=per_core[core]["onehotT"],
        )
        for l in range(2):
            m[f"wl{l}"] = wprep[f"wl{l}"]
            m[f"wr{l}"] = wprep[f"wr{l}"]
            m[f"aux{l}"] = wprep[f"aux{l}"]
        in_maps.append(m)

    trace = bool(inputs.pop("_trace", False))
    res = run_bass_kernel_spmd(nc, in_maps, core_ids=list(range(NCORES)),
                               trace=trace)
    out_rows = np.concatenate([res.results[c]["out"] for c in range(NCORES)],
                              axis=0)
    tmp = np.zeros((N, HC), np.float32)
    tmp[perm] = out_rows
    out = np.zeros((N, HC), np.float32)
    bias1 = np.asarray(inputs["bias1"], np.float32)
    out[:, col_perms[1]] = tmp * invs[1][None, :] + bias1[col_perms[1]][None, :]
    if trace:
        kernel._last_result = res
    return out
